# revision 8
# baseline (speedup 1.0000x reference)
"""Fused DDiT transformer block (causal) on 8 TRN2 NeuronCores.

Sharding: attention is head-parallel (2 heads/core, 16 total) with QKV
column-sliced per core; an AllToAll then re-shards from head-split to
token-split, and out-proj + MLP run token-parallel (512 tokens/core).
LayerNorm gains are folded into the following matmul weights on the host;
LN centering is folded into the matmuls via an appended K=1 rank-1 update
(-mu[t] * rowsum_w[e]) and the 1/std factor is folded into the RoPE tables
(q,k), a PSUM-eviction multiply (v), or a broadcast multiply (MLP).
Compute dtype bf16 (fp32 accumulation); the residual stream stays fp32.
"""
import sys

for _p in ("/opt/trn_rl_repo",):
    if _p not in sys.path:
        sys.path.append(_p)

import numpy as np
import ml_dtypes

import concourse.bass as bass
import concourse.tile as tile
import concourse.mybir as mybir
from concourse.bass_utils import run_bass_kernel_spmd
from concourse.masks import make_identity

bf16 = mybir.dt.bfloat16
f32 = mybir.dt.float32
AF = mybir.ActivationFunctionType
OP = mybir.AluOpType

N_CORES = 8
B, S, D = 2, 2048, 1024
T = B * S            # 4096 tokens total
NH, HD = 16, 64      # heads, head dim
HPC = NH // N_CORES  # 2 heads per core
TOK = T // N_CORES   # 512 tokens per core in the token-split phase
NT = T // 128        # 32 token tiles of 128
NCH = T // 512       # 8 chunks of 512 tokens
LN_EPS = 1e-5

# ---------------------------------------------------------------------------
# Sync legalizer: this walrus build accepts only ONE sync wait and ONE sync
# update per TPB instruction. Move extras onto same-engine NoOps (engines
# complete instructions in program order, so semantics are preserved).
# ---------------------------------------------------------------------------
_uid = [0]


def _legalize_sync(nc):
    for f in nc.m.functions:
        for bb in f.blocks:
            out = []
            changed = False
            for inst in bb.instructions:
                si = inst.sync_info
                if si is None:
                    out.append(inst)
                    continue
                waits = list(si.on_wait) if si.on_wait else []
                updates = list(si.on_update) if si.on_update else []
                if len(waits) <= 1 and len(updates) <= 1:
                    out.append(inst)
                    continue
                changed = True
                for w in waits[:-1]:
                    _uid[0] += 1
                    nop = mybir.InstNoOp(name=f"syncw-{_uid[0]}", ins=[], outs=[])
                    nop.engine = inst.engine
                    nop.sync_info = mybir.SyncInfo(on_wait=[w], on_update=[])
                    out.append(nop)
                inst.sync_info = mybir.SyncInfo(
                    on_wait=waits[-1:], on_update=updates[:1]
                )
                out.append(inst)
                for u in updates[1:]:
                    _uid[0] += 1
                    nop = mybir.InstNoOp(name=f"syncu-{_uid[0]}", ins=[], outs=[])
                    nop.engine = inst.engine
                    nop.sync_info = mybir.SyncInfo(on_wait=[], on_update=[u])
                    out.append(nop)
            if changed:
                bb.instructions = out
    return nc


# ---------------------------------------------------------------------------
# Kernel graph
# ---------------------------------------------------------------------------
def _build():
    nc = bass.Bass()

    # -- external inputs (per core)
    x_td = nc.dram_tensor("x_td", (T, D), bf16, kind="ExternalInput")
    xT_blk = nc.dram_tensor("xT_blk", (8, NCH, 128, 512), bf16, kind="ExternalInput")
    xT_own = nc.dram_tensor("xT_own", (D, TOK), f32, kind="ExternalInput")
    wqkv_blk = nc.dram_tensor("wqkv_blk", (3, 8, 128, 128), bf16, kind="ExternalInput")
    wqkv_rs = nc.dram_tensor("wqkv_rs", (3, 1, 128), bf16, kind="ExternalInput")
    tab = nc.dram_tensor("tab", (4, 128, T), bf16, kind="ExternalInput")  # cq, sq, ck, sk
    wout_blk = nc.dram_tensor("wout_blk", (8, 8, 128, 128), bf16, kind="ExternalInput")
    w1_blk = nc.dram_tensor("w1_blk", (32, 8, 128, 128), bf16, kind="ExternalInput")
    w1_rs = nc.dram_tensor("w1_rs", (32, 1, 128), bf16, kind="ExternalInput")
    b1_t = nc.dram_tensor("b1_t", (32, 128, 1), f32, kind="ExternalInput")
    w2_blk = nc.dram_tensor("w2_blk", (8, 32, 128, 128), bf16, kind="ExternalInput")
    b2_t = nc.dram_tensor("b2_t", (8, 128, 1), f32, kind="ExternalInput")
    out_d = nc.dram_tensor("out", (D, TOK), f32, kind="ExternalOutput")

    # -- internal DRAM
    stg_negmu = nc.dram_tensor("stg_negmu", (NT * 128,), bf16, kind="Internal")
    stg_rstd = nc.dram_tensor("stg_rstd", (NT * 128,), bf16, kind="Internal")
    cc_in = nc.dram_tensor("cc_in", (N_CORES, 128, TOK), bf16, kind="Internal")
    cc_out = nc.dram_tensor("cc_out", (N_CORES, 128, TOK), bf16, kind="Internal")

    with tile.TileContext(nc) as tc, \
         nc.allow_low_precision(reason="bf16 block compute"):
        with tc.tile_pool(name="const", bufs=1) as pconst, \
             tc.tile_pool(name="persist", bufs=1) as pper, \
             tc.tile_pool(name="stream", bufs=8) as pstream, \
             tc.tile_pool(name="big2", bufs=2) as pbig2, \
             tc.tile_pool(name="tabload", bufs=1) as ptab, \
             tc.tile_pool(name="work", bufs=3) as pwork:
            ident_bf = pconst.tile([128, 128], bf16)
            make_identity(nc, ident_bf)
            mask128 = pconst.tile([128, 128], bf16)
            nc.gpsimd.memset(mask128, 1.0)
            # causal: keep (1.0) where q_local - k_local = f - p >= 0
            nc.gpsimd.affine_select(
                out=mask128, in_=mask128, pattern=[[1, 128]],
                compare_op=OP.is_ge, fill=0.0, base=0, channel_multiplier=-1)
            ones_row = pconst.tile([1, 128], bf16)
            nc.vector.memset(ones_row, 1.0)
            ones_col = pconst.tile([128, 1], bf16)
            nc.vector.memset(ones_col, 1.0)
            ones65 = pconst.tile([65, 64], bf16)
            nc.vector.memset(ones65, 1.0)
            eps_col = pconst.tile([128, 1], f32)
            nc.vector.memset(eps_col, LN_EPS)

            # =============================================================
            # Phase A: LN1 statistics (bn_stats over [t, d] tiles)
            # =============================================================
            attn_pool_cm = tc.tile_pool(name="attn", bufs=1)
            pattn = attn_pool_cm.__enter__()
            stpack = pattn.tile([128, 64], bf16)  # cols 0:32 = -mu, 32:64 = 1/std
            for i in range(NT):
                xt = pwork.tile([128, D], bf16, tag="xstat")
                nc.sync.dma_start(out=xt, in_=x_td[i * 128:(i + 1) * 128, :])
                xg = xt.rearrange("p (g d) -> p g d", g=2)
                st6 = pwork.tile([128, 2, 6], f32, tag="st6")
                for g in range(2):
                    nc.vector.bn_stats(out=st6[:, g, :], in_=xg[:, g, :])
                mv = pwork.tile([128, 2], f32, tag="mv")
                nc.vector.bn_aggr(out=mv, in_=st6)
                sd = pwork.tile([128, 1], f32, tag="sd")
                nc.scalar.activation(out=sd, in_=mv[:, 1:2], func=AF.Sqrt, bias=eps_col)
                nc.vector.reciprocal(out=stpack[:, 32 + i:33 + i], in_=sd)
                nc.scalar.activation(out=stpack[:, i:i + 1], in_=mv[:, 0:1],
                                     func=AF.Copy, scale=-1.0)

            with tc.tile_pool(name="psA", bufs=1, space="PSUM") as psA:
                ps_st = psA.tile([64, 128], bf16)
                nc.tensor.transpose(out=ps_st, in_=stpack, identity=ident_bf)
                stT = pwork.tile([64, 128], bf16, tag="stT")
                nc.scalar.activation(out=stT, in_=ps_st, func=AF.Copy)
                nc.sync.dma_start(out=stg_negmu.rearrange("(a b) -> a b", b=128),
                                  in_=stT[0:32, :])
                nc.sync.dma_start(out=stg_rstd.rearrange("(a b) -> a b", b=128),
                                  in_=stT[32:64, :])

            negmu_row = pattn.tile([1, T], bf16)
            rstd_row = pattn.tile([1, T], bf16)
            nc.sync.dma_start(out=negmu_row[0:1, :], in_=stg_negmu[None, :])
            nc.sync.dma_start(out=rstd_row[0:1, :], in_=stg_rstd[None, :])

            # rstd broadcast to 128 partitions, all tokens
            rstd_sb = pattn.tile([128, T], bf16)
            with tc.tile_pool(name="psB", bufs=2, space="PSUM") as psB:
                for ch in range(NCH):
                    ps_b = psB.tile([128, 512], f32, tag="bc")
                    nc.tensor.matmul(ps_b, ones_row[0:1, 0:128],
                                     rstd_row[0:1, ch * 512:(ch + 1) * 512],
                                     start=True, stop=True)
                    nc.scalar.activation(out=rstd_sb[:, ch * 512:(ch + 1) * 512],
                                         in_=ps_b, func=AF.Copy)

            # rope tables folded with rstd
            tabs = []
            for ti in range(4):
                raw = ptab.tile([128, T], bf16, tag="tabraw")
                nc.sync.dma_start(out=raw, in_=tab[ti])
                eff = pattn.tile([128, T], bf16, name=f"tab{ti}", tag=f"tab{ti}")
                nc.vector.tensor_mul(out=eff, in0=raw, in1=rstd_sb)
                tabs.append(eff)
            tab_cq, tab_sq, tab_ck, tab_sk = tabs

            # persistent QKV weight tiles
            wq_sb = {}
            for m in range(3):
                for kk in range(8):
                    w = pconst.tile([128, 128], bf16, name=f"wqkv_{m}_{kk}", tag=f"wqkv_{m}_{kk}")
                    nc.sync.dma_start(out=w, in_=wqkv_blk[m, kk])
                    wq_sb[(m, kk)] = w
            rs_sb = {}
            for m in range(3):
                r = pconst.tile([1, 128], bf16, name=f"wqkvrs_{m}", tag=f"wqkvrs_{m}")
                nc.sync.dma_start(out=r, in_=wqkv_rs[m])
                rs_sb[m] = r

            # =============================================================
            # Phase B: QKV projection + RoPE + V transpose
            # =============================================================
            qT_sb = pattn.tile([128, T], bf16)
            kT_sb = pattn.tile([128, T], bf16)
            v_sb = [pattn.tile([128, 130], bf16, name=f"v_{g}", tag=f"v_{g}") for g in range(NT)]

            with tc.tile_pool(name="psQKV", bufs=3, space="PSUM") as psQ, \
                 tc.tile_pool(name="psVT", bufs=2, space="PSUM") as psVT:
                for ch in range(NCH):
                    sl = slice(ch * 512, (ch + 1) * 512)
                    xrt = pbig2.tile([128, 8, 512], bf16, tag="xTr")
                    for kk in range(8):
                        nc.sync.dma_start(out=xrt[:, kk, :], in_=xT_blk[kk, ch])
                    for m in range(3):
                        ps = psQ.tile([128, 512], f32, tag="qkv")
                        for kk in range(8):
                            nc.tensor.matmul(ps, wq_sb[(m, kk)], xrt[:, kk, :],
                                             start=(kk == 0), stop=False)
                        nc.tensor.matmul(ps, rs_sb[m], negmu_row[0:1, sl],
                                         start=False, stop=True)
                        if m < 2:  # q or k: rope
                            dst = qT_sb if m == 0 else kT_sb
                            tc_t = pwork.tile([128, 512], bf16, tag="ropec")
                            nc.scalar.activation(out=tc_t, in_=ps, func=AF.Copy)
                            tsw = pwork.tile([128, 512], bf16, tag="ropesw")
                            for h in range(2):
                                for a2 in range(2):
                                    nc.sync.dma_start(
                                        out=tsw[h * 64 + a2 * 32:h * 64 + a2 * 32 + 32, :],
                                        in_=tc_t[h * 64 + (1 - a2) * 32:h * 64 + (1 - a2) * 32 + 32, :])
                            tabc = tab_cq if m == 0 else tab_ck
                            tabs_ = tab_sq if m == 0 else tab_sk
                            t1 = pwork.tile([128, 512], bf16, tag="ropet1")
                            nc.vector.tensor_mul(out=t1, in0=tc_t, in1=tabc[:, sl])
                            t2 = pwork.tile([128, 512], bf16, tag="ropet2")
                            nc.vector.tensor_mul(out=t2, in0=tsw, in1=tabs_[:, sl])
                            nc.vector.tensor_add(out=dst[:, sl], in0=t1, in1=t2)
                        else:  # v: scale by rstd, transpose to [t, e] tiles
                            vt = pwork.tile([128, 512], bf16, tag="vtmp")
                            nc.vector.tensor_mul(out=vt, in0=ps, in1=rstd_sb[:, sl])
                            for j in range(4):
                                g = ch * 4 + j
                                pst = psVT.tile([128, 128], bf16, tag="vtr")
                                nc.tensor.transpose(out=pst, in_=vt[:, j * 128:(j + 1) * 128],
                                                    identity=ident_bf)
                                nc.vector.tensor_copy(out=v_sb[g][:, 0:64], in_=pst[:, 0:64])
                                nc.vector.tensor_copy(out=v_sb[g][:, 65:129], in_=pst[:, 64:128])
                                nc.vector.memset(v_sb[g][:, 64:65], 1.0)
                                nc.vector.memset(v_sb[g][:, 129:130], 1.0)

            # =============================================================
            # Phase C: causal attention per (batch, head), transposed layout
            # =============================================================
            with tc.tile_pool(name="psSC", bufs=3, space="PSUM") as psSC, \
                 tc.tile_pool(name="psO", bufs=2, space="PSUM") as psO, \
                 tc.tile_pool(name="psBC", bufs=2, space="PSUM") as psBC:
                for b in range(2):
                    for h in range(2):
                        hsl = slice(h * 64, (h + 1) * 64)
                        for qc in range(4):
                            qsl = slice(b * 2048 + qc * 512, b * 2048 + (qc + 1) * 512)
                            nkt = 4 * (qc + 1)
                            ps_o = psO.tile([65, 512], f32, tag="o")
                            for kt in range(nkt):
                                ps_s = psSC.tile([128, 512], f32, tag="sc")
                                ksl = slice(b * 2048 + kt * 128, b * 2048 + (kt + 1) * 128)
                                nc.tensor.matmul(ps_s, kT_sb[hsl, ksl], qT_sb[hsl, qsl],
                                                 start=True, stop=True)
                                p_t = pwork.tile([128, 512], bf16, tag="p")
                                nc.scalar.activation(out=p_t, in_=ps_s, func=AF.Exp)
                                if kt >= 4 * qc:  # diagonal block: causal mask
                                    off = kt * 128 - qc * 512
                                    if off > 0:
                                        nc.vector.memset(p_t[:, 0:off], 0.0)
                                    nc.vector.tensor_mul(
                                        out=p_t[:, off:off + 128],
                                        in0=p_t[:, off:off + 128], in1=mask128)
                                g = b * 16 + kt
                                nc.tensor.matmul(ps_o, v_sb[g][:, h * 65:(h + 1) * 65], p_t,
                                                 start=(kt == 0), stop=(kt == nkt - 1))
                            rec = pwork.tile([65, 512], bf16, tag="rec")
                            nc.vector.reciprocal(out=rec[64:65, :], in_=ps_o[64:65, :])
                            ps_b = psBC.tile([64, 512], f32, tag="ob")
                            nc.tensor.matmul(ps_b, ones65[64:65, :], rec[64:65, :],
                                             start=True, stop=True)
                            r64 = pwork.tile([64, 512], bf16, tag="r64")
                            nc.scalar.activation(out=r64, in_=ps_b, func=AF.Copy)
                            o_t = pwork.tile([64, 512], bf16, tag="o_t")
                            nc.vector.tensor_mul(out=o_t, in0=ps_o[0:64, :], in1=r64)
                            j = b * 4 + qc
                            nc.sync.dma_start(
                                out=cc_in[j, h * 64:(h + 1) * 64, :], in_=o_t)

            attn_pool_cm.__exit__(None, None, None)
            mlp_pool_cm = tc.tile_pool(name="mlp", bufs=1)
            pmlp = mlp_pool_cm.__enter__()

            # =============================================================
            # Phase D: AllToAll (head-split -> token-split), out-proj, LN2
            # =============================================================
            nc.gpsimd.collective_compute(
                "AllToAll", OP.bypass, ins=[cc_in[:, :, :]], outs=[cc_out[:, :, :]],
                replica_groups=[list(range(N_CORES))])

            o_own = [pmlp.tile([128, 512], bf16, name=f"oo_{kk}", tag=f"oo_{kk}") for kk in range(8)]
            xT_o = [pmlp.tile([128, 512], f32, name=f"xo_{kk}", tag=f"xo_{kk}") for kk in range(8)]
            for kk in range(8):
                nc.sync.dma_start(out=o_own[kk], in_=cc_out[kk])
                nc.sync.dma_start(out=xT_o[kk], in_=xT_own[kk * 128:(kk + 1) * 128, :])

            xa = [pmlp.tile([128, 512], f32, name=f"xa_{m}", tag=f"xa_{m}") for m in range(8)]
            xab = [pmlp.tile([128, 512], bf16, name=f"xab_{m}", tag=f"xab_{m}") for m in range(8)]
            with tc.tile_pool(name="psOP", bufs=2, space="PSUM") as psOP, \
                 tc.tile_pool(name="psMU", bufs=1, space="PSUM") as psMU, \
                 tc.tile_pool(name="psSQ", bufs=1, space="PSUM") as psSQ, \
                 tc.tile_pool(name="psRB", bufs=1, space="PSUM") as psRB:
                ps_mu = psMU.tile([1, 512], f32)
                ps_sq = psSQ.tile([1, 512], f32)
                for m in range(8):
                    ps = psOP.tile([128, 512], f32, tag="op")
                    for kk in range(8):
                        w = pstream.tile([128, 128], bf16, tag="wo_st")
                        nc.sync.dma_start(out=w, in_=wout_blk[m, kk])
                        nc.tensor.matmul(ps, w, o_own[kk], start=(kk == 0), stop=(kk == 7))
                    nc.vector.tensor_add(out=xa[m], in0=ps, in1=xT_o[m])
                    nc.scalar.activation(out=xab[m], in_=xa[m], func=AF.Copy)
                    sq = pwork.tile([128, 512], bf16, tag="sq")
                    nc.scalar.activation(out=sq, in_=xab[m], func=AF.Square)
                    nc.tensor.matmul(ps_mu, ones_col, xab[m],
                                     start=(m == 0), stop=(m == 7))
                    nc.tensor.matmul(ps_sq, ones_col, sq,
                                     start=(m == 0), stop=(m == 7))

                # LN2 row stats: mu = sum/1024, var = sqsum/1024 - mu^2
                mu_r = pwork.tile([1, 512], f32, tag="mu_r")
                nc.scalar.activation(out=mu_r, in_=ps_mu, func=AF.Copy, scale=1.0 / D)
                var_r = pwork.tile([1, 512], f32, tag="var_r")
                nc.scalar.activation(out=var_r, in_=ps_sq, func=AF.Copy, scale=1.0 / D)
                mus_r = pwork.tile([1, 512], f32, tag="mus_r")
                nc.vector.tensor_mul(out=mus_r, in0=mu_r, in1=mu_r)
                nc.vector.tensor_sub(out=var_r, in0=var_r, in1=mus_r)
                sd_r = pwork.tile([1, 512], f32, tag="sd_r")
                nc.scalar.activation(out=sd_r, in_=var_r, func=AF.Sqrt, bias=eps_col[0:1, :])
                rstd2 = pmlp.tile([1, 512], bf16)
                nc.vector.reciprocal(out=rstd2, in_=sd_r)
                negmu2 = pmlp.tile([1, 512], bf16)
                nc.scalar.activation(out=negmu2, in_=mu_r, func=AF.Copy, scale=-1.0)
                ps_rb = psRB.tile([128, 512], f32)
                nc.tensor.matmul(ps_rb, ones_row[0:1, 0:128], rstd2, start=True, stop=True)
                rstd2_sb = pmlp.tile([128, 512], bf16)
                nc.scalar.activation(out=rstd2_sb, in_=ps_rb, func=AF.Copy)

            # =============================================================
            # Phase E: MLP (token-split, full weights)
            # =============================================================
            u_g = [pmlp.tile([128, 512], bf16, name=f"ug_{m}", tag=f"ug_{m}") for m in range(32)]
            with tc.tile_pool(name="psU", bufs=3, space="PSUM") as psU, \
                 tc.tile_pool(name="psDn", bufs=2, space="PSUM") as psDn:
                for m in range(32):
                    ps = psU.tile([128, 512], f32, tag="u")
                    for kk in range(8):
                        w = pstream.tile([128, 128], bf16, tag="w1_st")
                        nc.sync.dma_start(out=w, in_=w1_blk[m, kk])
                        nc.tensor.matmul(ps, w, xab[kk], start=(kk == 0), stop=False)
                    r1 = pwork.tile([1, 128], bf16, tag="w1rs_st")
                    nc.sync.dma_start(out=r1, in_=w1_rs[m])
                    nc.tensor.matmul(ps, r1, negmu2, start=False, stop=True)
                    upre = pwork.tile([128, 512], bf16, tag="upre")
                    nc.vector.tensor_mul(out=upre, in0=ps, in1=rstd2_sb)
                    b1 = pwork.tile([128, 1], f32, tag="b1_st")
                    nc.sync.dma_start(out=b1, in_=b1_t[m])
                    nc.scalar.activation(out=u_g[m], in_=upre,
                                         func=AF.Gelu_apprx_tanh, bias=b1)
                for m in range(8):
                    ps = psDn.tile([128, 512], f32, tag="dn")
                    for kk in range(32):
                        w = pstream.tile([128, 128], bf16, tag="w2_st")
                        nc.sync.dma_start(out=w, in_=w2_blk[m, kk])
                        nc.tensor.matmul(ps, w, u_g[kk], start=(kk == 0), stop=(kk == 31))
                    b2 = pwork.tile([128, 1], f32, tag="b2_st")
                    nc.sync.dma_start(out=b2, in_=b2_t[m])
                    mt = pwork.tile([128, 512], f32, tag="mt")
                    nc.scalar.activation(out=mt, in_=ps, func=AF.Identity, bias=b2)
                    ot = pwork.tile([128, 512], f32, tag="ot")
                    nc.vector.tensor_add(out=ot, in0=mt, in1=xa[m])
                    nc.sync.dma_start(out=out_d[m * 128:(m + 1) * 128, :], in_=ot)
            mlp_pool_cm.__exit__(None, None, None)

    _legalize_sync(nc)
    return nc


# ---------------------------------------------------------------------------
# Host-side prep + execution
# ---------------------------------------------------------------------------
_NC_CACHE = {}


def _get_nc():
    if "nc" not in _NC_CACHE:
        _NC_CACHE["nc"] = _build()
    return _NC_CACHE["nc"]


def _bf(a):
    return np.ascontiguousarray(a).astype(ml_dtypes.bfloat16)


def _f32(a):
    return np.ascontiguousarray(a, dtype=np.float32)


def _prep_inputs(x, rot_cos, rot_sin, ln1_w, w_qkv, w_out, ln2_w, w_mlp1,
                 b_mlp1, w_mlp2, b_mlp2):
    x = np.asarray(x, np.float32)
    X = x.reshape(T, D)

    x_td = _bf(X)
    xT = X.T  # (D, T)
    xT_blk = _bf(xT.reshape(8, 128, NCH, 512).transpose(0, 2, 1, 3))

    # rope tables: (128 rows = 2 heads x [first32|last32]) x T tokens
    cos = np.asarray(rot_cos, np.float32)[0, :, 0, 0, :HD // 2]  # (S, 32)
    sin = np.asarray(rot_sin, np.float32)[0, :, 0, 0, :HD // 2]
    cT = np.concatenate([cos, cos], 1).T          # (64, S)
    sT = np.concatenate([-sin, sin], 1).T         # (64, S) sign-folded
    cT = np.tile(cT, (2, B))                      # (128, T)
    sT = np.tile(sT, (2, B))
    tab = _bf(np.stack([cT * 0.125, sT * 0.125, cT, sT]))  # cq, sq, ck, sk

    wqkv_eff = np.asarray(w_qkv, np.float32) * np.asarray(ln1_w, np.float32)[None, :]
    w1_eff = np.asarray(w_mlp1, np.float32) * np.asarray(ln2_w, np.float32)[None, :]
    w_out_f = np.asarray(w_out, np.float32)
    w2_f = np.asarray(w_mlp2, np.float32)

    woutT = w_out_f.T  # (d_in=head dims, e)
    wout_blk = _bf(woutT.reshape(8, 128, 8, 128).transpose(2, 0, 1, 3))  # [m, kk]
    w1T = w1_eff.T     # (D, 4D)
    w1_blk = _bf(w1T.reshape(8, 128, 32, 128).transpose(2, 0, 1, 3))
    w1_rs = _bf(w1_eff.sum(1).reshape(32, 1, 128))
    w2T = w2_f.T       # (4D, D)
    w2_blk = _bf(w2T.reshape(32, 128, 8, 128).transpose(2, 0, 1, 3))
    b1_arr = _f32(np.asarray(b_mlp1, np.float32).reshape(32, 128, 1))
    b2_arr = _f32(np.asarray(b_mlp2, np.float32).reshape(8, 128, 1))

    in_maps = []
    for c in range(N_CORES):
        w_sl = np.concatenate(
            [wqkv_eff[0 * D + 2 * c * HD: 0 * D + 2 * (c + 1) * HD],
             wqkv_eff[1 * D + 2 * c * HD: 1 * D + 2 * (c + 1) * HD],
             wqkv_eff[2 * D + 2 * c * HD: 2 * D + 2 * (c + 1) * HD]], 0)  # (384, D)
        wT_sl = w_sl.T  # (D, 384) -> [m, kk, 128, 128]
        wqkv_b = _bf(wT_sl.reshape(8, 128, 3, 128).transpose(2, 0, 1, 3))
        wqkv_rsum = _bf(w_sl.sum(1).reshape(3, 1, 128))
        in_maps.append({
            "x_td": x_td,
            "xT_blk": xT_blk,
            "xT_own": _f32(xT[:, c * TOK:(c + 1) * TOK]),
            "wqkv_blk": wqkv_b,
            "wqkv_rs": wqkv_rsum,
            "tab": tab,
            "wout_blk": wout_blk,
            "w1_blk": w1_blk,
            "w1_rs": w1_rs,
            "b1_t": b1_arr,
            "w2_blk": w2_blk,
            "b2_t": b2_arr,
        })
    return in_maps


def _assemble(results):
    outT = np.concatenate([results[c]["out"] for c in range(N_CORES)], axis=1)
    return np.ascontiguousarray(outT.T.astype(np.float32)).reshape(B, S, D)


def run_spmd(in_maps, **kwargs):
    nc = _get_nc()
    return run_bass_kernel_spmd(nc, in_maps, core_ids=list(range(N_CORES)), **kwargs)


def kernel(x, rot_cos, rot_sin, ln1_w, w_qkv, w_out, ln2_w, w_mlp1, b_mlp1,
           w_mlp2, b_mlp2):
    in_maps = _prep_inputs(x, rot_cos, rot_sin, ln1_w, w_qkv, w_out, ln2_w,
                           w_mlp1, b_mlp1, w_mlp2, b_mlp2)
    res = run_spmd(in_maps)
    return _assemble(res.results)


# revision 12
# speedup vs baseline: 1.3694x; 1.3694x over previous
"""Fused DDiT transformer block (causal) on 8 TRN2 NeuronCores.

Sharding: attention is head-parallel (2 heads/core, 16 total) with QKV
column-sliced per core; an AllToAll then re-shards from head-split to
token-split, and out-proj + MLP run token-parallel (512 tokens/core).
LayerNorm gains are folded into the following matmul weights on the host;
LN centering is folded into the matmuls via an appended K=1 rank-1 update
(-mu[t] * rowsum_w[e]) and the 1/std factor is folded into the RoPE tables
(q,k), a PSUM-eviction multiply (v), or a broadcast multiply (MLP).
Compute dtype bf16 (fp32 accumulation); the residual stream stays fp32.
"""
import sys

for _p in ("/opt/trn_rl_repo",):
    if _p not in sys.path:
        sys.path.append(_p)

import numpy as np
import ml_dtypes

import concourse.bass as bass
import concourse.tile as tile
import concourse.mybir as mybir
from concourse.bass_utils import run_bass_kernel_spmd
from concourse.masks import make_identity

bf16 = mybir.dt.bfloat16
f32 = mybir.dt.float32
AF = mybir.ActivationFunctionType
OP = mybir.AluOpType

N_CORES = 8
B, S, D = 2, 2048, 1024
T = B * S            # 4096 tokens total
NH, HD = 16, 64      # heads, head dim
HPC = NH // N_CORES  # 2 heads per core
TOK = T // N_CORES   # 512 tokens per core in the token-split phase
NT = T // 128        # 32 token tiles of 128
NCH = T // 512       # 8 chunks of 512 tokens
LN_EPS = 1e-5

# ---------------------------------------------------------------------------
# Sync legalizer: this walrus build accepts only ONE sync wait and ONE sync
# update per TPB instruction. Move extras onto same-engine NoOps (engines
# complete instructions in program order, so semantics are preserved).
# ---------------------------------------------------------------------------
_uid = [0]


def _legalize_sync(nc):
    for f in nc.m.functions:
        for bb in f.blocks:
            out = []
            changed = False
            for inst in bb.instructions:
                si = inst.sync_info
                if si is None:
                    out.append(inst)
                    continue
                waits = list(si.on_wait) if si.on_wait else []
                updates = list(si.on_update) if si.on_update else []
                if len(waits) <= 1 and len(updates) <= 1:
                    out.append(inst)
                    continue
                changed = True
                for w in waits[:-1]:
                    _uid[0] += 1
                    nop = mybir.InstNoOp(name=f"syncw-{_uid[0]}", ins=[], outs=[])
                    nop.engine = inst.engine
                    nop.sync_info = mybir.SyncInfo(on_wait=[w], on_update=[])
                    out.append(nop)
                inst.sync_info = mybir.SyncInfo(
                    on_wait=waits[-1:], on_update=updates[:1]
                )
                out.append(inst)
                for u in updates[1:]:
                    _uid[0] += 1
                    nop = mybir.InstNoOp(name=f"syncu-{_uid[0]}", ins=[], outs=[])
                    nop.engine = inst.engine
                    nop.sync_info = mybir.SyncInfo(on_wait=[], on_update=[u])
                    out.append(nop)
            if changed:
                bb.instructions = out
    return nc


# ---------------------------------------------------------------------------
# Kernel graph
# ---------------------------------------------------------------------------
def _build():
    nc = bass.Bass()

    # -- external inputs (per core)
    x_td = nc.dram_tensor("x_td", (T, D), bf16, kind="ExternalInput")
    xT_blk = nc.dram_tensor("xT_blk", (NCH, 128, 8, 512), bf16, kind="ExternalInput")
    xT_own = nc.dram_tensor("xT_own", (D, TOK), f32, kind="ExternalInput")
    wqkv_blk = nc.dram_tensor("wqkv_blk", (3, 128, 8, 128), bf16, kind="ExternalInput")
    wqkv_rs = nc.dram_tensor("wqkv_rs", (3, 1, 128), bf16, kind="ExternalInput")
    tab = nc.dram_tensor("tab", (2, 128, T), bf16, kind="ExternalInput")  # cos, sin(signed)
    wout_blk = nc.dram_tensor("wout_blk", (8, 128, 8, 128), bf16, kind="ExternalInput")
    w1_blk = nc.dram_tensor("w1_blk", (32, 128, 8, 128), bf16, kind="ExternalInput")
    w1_rs = nc.dram_tensor("w1_rs", (32, 1, 128), bf16, kind="ExternalInput")
    b1_t = nc.dram_tensor("b1_t", (32, 128, 1), f32, kind="ExternalInput")
    w2_blk = nc.dram_tensor("w2_blk", (8, 128, 32, 128), bf16, kind="ExternalInput")
    b2_t = nc.dram_tensor("b2_t", (8, 128, 1), f32, kind="ExternalInput")
    out_d = nc.dram_tensor("out", (D, TOK), f32, kind="ExternalOutput")

    # -- internal DRAM
    stg_negmu = nc.dram_tensor("stg_negmu", (NT * 128,), bf16, kind="Internal")
    stg_rstd = nc.dram_tensor("stg_rstd", (NT * 128,), bf16, kind="Internal")
    cc_in = nc.dram_tensor("cc_in", (N_CORES, 128, TOK), bf16, kind="Internal")
    cc_out = nc.dram_tensor("cc_out", (N_CORES, 128, TOK), bf16, kind="Internal")

    with tile.TileContext(nc) as tc, \
         nc.allow_low_precision(reason="bf16 block compute"):
        with tc.tile_pool(name="const", bufs=1) as pconst, \
             tc.tile_pool(name="persist", bufs=1) as pper, \
             tc.tile_pool(name="stream", bufs=2) as pstream, \
             tc.tile_pool(name="big2", bufs=2) as pbig2, \
             tc.tile_pool(name="tabload", bufs=1) as ptab, \
             tc.tile_pool(name="work", bufs=3) as pwork:
            ident_bf = pconst.tile([128, 128], bf16)
            make_identity(nc, ident_bf)
            mask128 = pconst.tile([128, 128], bf16)
            nc.gpsimd.memset(mask128, 1.0)
            # causal: keep (1.0) where q_local - k_local = f - p >= 0
            nc.gpsimd.affine_select(
                out=mask128, in_=mask128, pattern=[[1, 128]],
                compare_op=OP.is_ge, fill=0.0, base=0, channel_multiplier=-1)
            ones_row = pconst.tile([1, 128], bf16)
            nc.vector.memset(ones_row, 1.0)
            ones_col = pconst.tile([128, 1], bf16)
            nc.vector.memset(ones_col, 1.0)
            ones65 = pconst.tile([65, 64], bf16)
            nc.vector.memset(ones65, 1.0)
            eps_col = pconst.tile([128, 1], f32)
            nc.vector.memset(eps_col, LN_EPS)

            # =============================================================
            # Phase A: LN1 statistics (bn_stats over [t, d] tiles)
            # =============================================================
            attn_pool_cm = tc.tile_pool(name="attn", bufs=1)
            pattn = attn_pool_cm.__enter__()
            stpack = pattn.tile([128, 64], bf16)  # cols 0:32 = -mu, 32:64 = 1/std
            for i in range(NT):
                xt = pwork.tile([128, D], bf16, tag="xstat", bufs=2)
                nc.sync.dma_start(out=xt, in_=x_td[i * 128:(i + 1) * 128, :])
                xg = xt.rearrange("p (g d) -> p g d", g=2)
                st6 = pwork.tile([128, 2, 6], f32, tag="st6")
                for g in range(2):
                    nc.vector.bn_stats(out=st6[:, g, :], in_=xg[:, g, :])
                mv = pwork.tile([128, 2], f32, tag="mv")
                nc.vector.bn_aggr(out=mv, in_=st6)
                sd = pwork.tile([128, 1], f32, tag="sd")
                nc.scalar.activation(out=sd, in_=mv[:, 1:2], func=AF.Sqrt, bias=eps_col)
                nc.vector.reciprocal(out=stpack[:, 32 + i:33 + i], in_=sd)
                nc.scalar.activation(out=stpack[:, i:i + 1], in_=mv[:, 0:1],
                                     func=AF.Copy, scale=-1.0)

            with tc.tile_pool(name="psA", bufs=1, space="PSUM") as psA:
                ps_st = psA.tile([64, 128], bf16)
                nc.tensor.transpose(out=ps_st, in_=stpack, identity=ident_bf)
                stT = pwork.tile([64, 128], bf16, tag="stT")
                nc.scalar.activation(out=stT, in_=ps_st, func=AF.Copy)
                nc.sync.dma_start(out=stg_negmu.rearrange("(a b) -> a b", b=128),
                                  in_=stT[0:32, :])
                nc.sync.dma_start(out=stg_rstd.rearrange("(a b) -> a b", b=128),
                                  in_=stT[32:64, :])

            negmu_row = pattn.tile([1, T], bf16)
            rstd_row = pattn.tile([1, T], bf16)
            nc.sync.dma_start(out=negmu_row[0:1, :], in_=stg_negmu[None, :])
            nc.sync.dma_start(out=rstd_row[0:1, :], in_=stg_rstd[None, :])

            # rstd broadcast to 128 partitions, all tokens
            rstd_sb = pattn.tile([128, T], bf16)
            with tc.tile_pool(name="psB", bufs=2, space="PSUM") as psB:
                for ch in range(NCH):
                    ps_b = psB.tile([128, 512], f32, tag="bc")
                    nc.tensor.matmul(ps_b, ones_row[0:1, 0:128],
                                     rstd_row[0:1, ch * 512:(ch + 1) * 512],
                                     start=True, stop=True)
                    nc.scalar.activation(out=rstd_sb[:, ch * 512:(ch + 1) * 512],
                                         in_=ps_b, func=AF.Copy)

            # rope tables folded with rstd (1/sqrt(hd) is folded into w_q)
            tabs = []
            for ti in range(2):
                raw = ptab.tile([128, T], bf16, tag="tabraw")
                nc.sync.dma_start(out=raw, in_=tab[ti])
                eff = pattn.tile([128, T], bf16, name=f"tab{ti}", tag=f"tab{ti}")
                nc.vector.tensor_mul(out=eff, in0=raw, in1=rstd_sb)
                tabs.append(eff)
            tab_c, tab_s = tabs

            # persistent QKV weight tiles (one packed tile per m)
            wq_sb = {}
            for m in range(3):
                w = pconst.tile([128, 8, 128], bf16, name=f"wqkv_{m}", tag=f"wqkv_{m}")
                nc.sync.dma_start(out=w, in_=wqkv_blk[m])
                wq_sb[m] = w
            rs_sb = {}
            for m in range(3):
                r = pconst.tile([1, 128], bf16, name=f"wqkvrs_{m}", tag=f"wqkvrs_{m}")
                nc.sync.dma_start(out=r, in_=wqkv_rs[m])
                rs_sb[m] = r

            # =============================================================
            # Phase B: QKV projection + RoPE + V transpose
            # =============================================================
            qT_sb = pattn.tile([128, T], bf16)
            kT_sb = pattn.tile([128, T], bf16)
            v_sb = [pattn.tile([128, 130], bf16, name=f"v_{g}", tag=f"v_{g}") for g in range(NT)]

            with tc.tile_pool(name="psQKV", bufs=3, space="PSUM") as psQ, \
                 tc.tile_pool(name="psVT", bufs=2, space="PSUM") as psVT:
                for ch in range(NCH):
                    sl = slice(ch * 512, (ch + 1) * 512)
                    xrt = pbig2.tile([128, 8, 512], bf16, tag="xTr")
                    nc.sync.dma_start(out=xrt, in_=xT_blk[ch])
                    for m in range(3):
                        ps = psQ.tile([128, 512], f32, tag="qkv")
                        for kk in range(8):
                            nc.tensor.matmul(ps, wq_sb[m][:, kk, :], xrt[:, kk, :],
                                             start=(kk == 0), stop=False)
                        nc.tensor.matmul(ps, rs_sb[m], negmu_row[0:1, sl],
                                         start=False, stop=True)
                        if m < 2:  # q or k: rope
                            dst = qT_sb if m == 0 else kT_sb
                            tc_t = pwork.tile([128, 512], bf16, tag="ropec", bufs=2)
                            nc.scalar.activation(out=tc_t, in_=ps, func=AF.Copy)
                            tsw = pwork.tile([128, 512], bf16, tag="ropesw", bufs=2)
                            for h in range(2):
                                for a2 in range(2):
                                    nc.sync.dma_start(
                                        out=tsw[h * 64 + a2 * 32:h * 64 + a2 * 32 + 32, :],
                                        in_=tc_t[h * 64 + (1 - a2) * 32:h * 64 + (1 - a2) * 32 + 32, :])
                            tabc, tabs_ = tab_c, tab_s
                            t1 = pwork.tile([128, 512], bf16, tag="ropet1", bufs=2)
                            nc.vector.tensor_mul(out=t1, in0=tc_t, in1=tabc[:, sl])
                            t2 = pwork.tile([128, 512], bf16, tag="ropet2", bufs=2)
                            nc.vector.tensor_mul(out=t2, in0=tsw, in1=tabs_[:, sl])
                            nc.vector.tensor_add(out=dst[:, sl], in0=t1, in1=t2)
                        else:  # v: scale by rstd, transpose to [t, e] tiles
                            vt = pwork.tile([128, 512], bf16, tag="vtmp")
                            nc.vector.tensor_mul(out=vt, in0=ps, in1=rstd_sb[:, sl])
                            for j in range(4):
                                g = ch * 4 + j
                                pst = psVT.tile([128, 128], bf16, tag="vtr")
                                nc.tensor.transpose(out=pst, in_=vt[:, j * 128:(j + 1) * 128],
                                                    identity=ident_bf)
                                nc.vector.tensor_copy(out=v_sb[g][:, 0:64], in_=pst[:, 0:64])
                                nc.vector.tensor_copy(out=v_sb[g][:, 65:129], in_=pst[:, 64:128])
                                nc.vector.memset(v_sb[g][:, 64:65], 1.0)
                                nc.vector.memset(v_sb[g][:, 129:130], 1.0)

            # =============================================================
            # Phase C: causal attention per (batch, head), transposed layout
            # =============================================================
            with tc.tile_pool(name="psSC", bufs=3, space="PSUM") as psSC, \
                 tc.tile_pool(name="psO", bufs=2, space="PSUM") as psO, \
                 tc.tile_pool(name="psBC", bufs=2, space="PSUM") as psBC:
                for b in range(2):
                    for h in range(2):
                        hsl = slice(h * 64, (h + 1) * 64)
                        for qc in range(4):
                            qsl = slice(b * 2048 + qc * 512, b * 2048 + (qc + 1) * 512)
                            nkt = 4 * (qc + 1)
                            ps_o = psO.tile([65, 512], f32, tag="o")
                            for kt in range(nkt):
                                ps_s = psSC.tile([128, 512], f32, tag="sc")
                                ksl = slice(b * 2048 + kt * 128, b * 2048 + (kt + 1) * 128)
                                nc.tensor.matmul(ps_s, kT_sb[hsl, ksl], qT_sb[hsl, qsl],
                                                 start=True, stop=True)
                                p_t = pwork.tile([128, 512], bf16, tag="p")
                                nc.scalar.activation(out=p_t, in_=ps_s, func=AF.Exp)
                                if kt >= 4 * qc:  # diagonal block: causal mask
                                    off = kt * 128 - qc * 512
                                    if off > 0:
                                        nc.vector.memset(p_t[:, 0:off], 0.0)
                                    nc.vector.tensor_mul(
                                        out=p_t[:, off:off + 128],
                                        in0=p_t[:, off:off + 128], in1=mask128)
                                g = b * 16 + kt
                                nc.tensor.matmul(ps_o, v_sb[g][:, h * 65:(h + 1) * 65], p_t,
                                                 start=(kt == 0), stop=(kt == nkt - 1))
                            rec = pwork.tile([65, 512], bf16, tag="rec")
                            nc.vector.reciprocal(out=rec[64:65, :], in_=ps_o[64:65, :])
                            ps_b = psBC.tile([64, 512], f32, tag="ob")
                            nc.tensor.matmul(ps_b, ones65[64:65, :], rec[64:65, :],
                                             start=True, stop=True)
                            r64 = pwork.tile([64, 512], bf16, tag="r64")
                            nc.scalar.activation(out=r64, in_=ps_b, func=AF.Copy)
                            o_t = pwork.tile([64, 512], bf16, tag="o_t")
                            nc.vector.tensor_mul(out=o_t, in0=ps_o[0:64, :], in1=r64)
                            j = b * 4 + qc
                            nc.sync.dma_start(
                                out=cc_in[j, h * 64:(h + 1) * 64, :], in_=o_t)

            attn_pool_cm.__exit__(None, None, None)
            mlp_pool_cm = tc.tile_pool(name="mlp", bufs=1)
            pmlp = mlp_pool_cm.__enter__()

            # =============================================================
            # Phase D: AllToAll (head-split -> token-split), out-proj, LN2
            # =============================================================
            nc.gpsimd.collective_compute(
                "AllToAll", OP.bypass, ins=[cc_in[:, :, :]], outs=[cc_out[:, :, :]],
                replica_groups=[list(range(N_CORES))])

            o_own = [pmlp.tile([128, 512], bf16, name=f"oo_{kk}", tag=f"oo_{kk}") for kk in range(8)]
            xT_o = [pmlp.tile([128, 512], f32, name=f"xo_{kk}", tag=f"xo_{kk}") for kk in range(8)]
            for kk in range(8):
                nc.sync.dma_start(out=o_own[kk], in_=cc_out[kk])
                nc.sync.dma_start(out=xT_o[kk], in_=xT_own[kk * 128:(kk + 1) * 128, :])

            xa = [pmlp.tile([128, 512], f32, name=f"xa_{m}", tag=f"xa_{m}") for m in range(8)]
            xab = [pmlp.tile([128, 512], bf16, name=f"xab_{m}", tag=f"xab_{m}") for m in range(8)]
            with tc.tile_pool(name="psOP", bufs=2, space="PSUM") as psOP, \
                 tc.tile_pool(name="psMU", bufs=1, space="PSUM") as psMU, \
                 tc.tile_pool(name="psSQ", bufs=1, space="PSUM") as psSQ, \
                 tc.tile_pool(name="psRB", bufs=1, space="PSUM") as psRB:
                ps_mu = psMU.tile([1, 512], f32)
                ps_sq = psSQ.tile([1, 512], f32)
                for m in range(8):
                    ps = psOP.tile([128, 512], f32, tag="op")
                    w = pstream.tile([128, 8, 128], bf16, tag="wo_st")
                    nc.sync.dma_start(out=w, in_=wout_blk[m])
                    for kk in range(8):
                        nc.tensor.matmul(ps, w[:, kk, :], o_own[kk], start=(kk == 0), stop=(kk == 7))
                    nc.vector.tensor_add(out=xa[m], in0=ps, in1=xT_o[m])
                    nc.scalar.activation(out=xab[m], in_=xa[m], func=AF.Copy)
                    sq = pwork.tile([128, 512], bf16, tag="sq")
                    nc.scalar.activation(out=sq, in_=xab[m], func=AF.Square)
                    nc.tensor.matmul(ps_mu, ones_col, xab[m],
                                     start=(m == 0), stop=(m == 7))
                    nc.tensor.matmul(ps_sq, ones_col, sq,
                                     start=(m == 0), stop=(m == 7))

                # LN2 row stats: mu = sum/1024, var = sqsum/1024 - mu^2
                mu_r = pwork.tile([1, 512], f32, tag="mu_r")
                nc.scalar.activation(out=mu_r, in_=ps_mu, func=AF.Copy, scale=1.0 / D)
                var_r = pwork.tile([1, 512], f32, tag="var_r")
                nc.scalar.activation(out=var_r, in_=ps_sq, func=AF.Copy, scale=1.0 / D)
                mus_r = pwork.tile([1, 512], f32, tag="mus_r")
                nc.vector.tensor_mul(out=mus_r, in0=mu_r, in1=mu_r)
                nc.vector.tensor_sub(out=var_r, in0=var_r, in1=mus_r)
                sd_r = pwork.tile([1, 512], f32, tag="sd_r")
                nc.scalar.activation(out=sd_r, in_=var_r, func=AF.Sqrt, bias=eps_col[0:1, :])
                rstd2 = pmlp.tile([1, 512], bf16)
                nc.vector.reciprocal(out=rstd2, in_=sd_r)
                negmu2 = pmlp.tile([1, 512], bf16)
                nc.scalar.activation(out=negmu2, in_=mu_r, func=AF.Copy, scale=-1.0)
                ps_rb = psRB.tile([128, 512], f32)
                nc.tensor.matmul(ps_rb, ones_row[0:1, 0:128], rstd2, start=True, stop=True)
                rstd2_sb = pmlp.tile([128, 512], bf16)
                nc.scalar.activation(out=rstd2_sb, in_=ps_rb, func=AF.Copy)

            # =============================================================
            # Phase E: MLP (token-split, full weights)
            # =============================================================
            u_g = [pmlp.tile([128, 512], bf16, name=f"ug_{m}", tag=f"ug_{m}") for m in range(32)]
            with tc.tile_pool(name="psU", bufs=3, space="PSUM") as psU, \
                 tc.tile_pool(name="psDn", bufs=2, space="PSUM") as psDn:
                for m in range(32):
                    ps = psU.tile([128, 512], f32, tag="u")
                    w = pstream.tile([128, 8, 128], bf16, tag="w1_st")
                    nc.sync.dma_start(out=w, in_=w1_blk[m])
                    for kk in range(8):
                        nc.tensor.matmul(ps, w[:, kk, :], xab[kk], start=(kk == 0), stop=False)
                    r1 = pwork.tile([1, 128], bf16, tag="w1rs_st")
                    nc.sync.dma_start(out=r1, in_=w1_rs[m])
                    nc.tensor.matmul(ps, r1, negmu2, start=False, stop=True)
                    upre = pwork.tile([128, 512], bf16, tag="upre")
                    nc.vector.tensor_mul(out=upre, in0=ps, in1=rstd2_sb)
                    b1 = pwork.tile([128, 1], f32, tag="b1_st")
                    nc.sync.dma_start(out=b1, in_=b1_t[m])
                    nc.scalar.activation(out=u_g[m], in_=upre,
                                         func=AF.Gelu_apprx_tanh, bias=b1)
                for m in range(8):
                    ps = psDn.tile([128, 512], f32, tag="dn")
                    w = pstream.tile([128, 32, 128], bf16, tag="w2_st")
                    nc.sync.dma_start(out=w, in_=w2_blk[m])
                    for kk in range(32):
                        nc.tensor.matmul(ps, w[:, kk, :], u_g[kk], start=(kk == 0), stop=(kk == 31))
                    b2 = pwork.tile([128, 1], f32, tag="b2_st")
                    nc.sync.dma_start(out=b2, in_=b2_t[m])
                    mt = pwork.tile([128, 512], f32, tag="mt", bufs=2)
                    nc.scalar.activation(out=mt, in_=ps, func=AF.Identity, bias=b2)
                    ot = pwork.tile([128, 512], f32, tag="ot", bufs=2)
                    nc.vector.tensor_add(out=ot, in0=mt, in1=xa[m])
                    nc.sync.dma_start(out=out_d[m * 128:(m + 1) * 128, :], in_=ot)
            mlp_pool_cm.__exit__(None, None, None)

    _legalize_sync(nc)
    return nc


# ---------------------------------------------------------------------------
# Host-side prep + execution
# ---------------------------------------------------------------------------
_NC_CACHE = {}


def _get_nc():
    if "nc" not in _NC_CACHE:
        _NC_CACHE["nc"] = _build()
    return _NC_CACHE["nc"]


def _bf(a):
    return np.ascontiguousarray(a).astype(ml_dtypes.bfloat16)


def _f32(a):
    return np.ascontiguousarray(a, dtype=np.float32)


def _prep_inputs(x, rot_cos, rot_sin, ln1_w, w_qkv, w_out, ln2_w, w_mlp1,
                 b_mlp1, w_mlp2, b_mlp2):
    x = np.asarray(x, np.float32)
    X = x.reshape(T, D)

    x_td = _bf(X)
    xT = X.T  # (D, T)
    # (ch, p, kk, t): partition row p holds all kk-blocks contiguously
    xT_blk = _bf(xT.reshape(8, 128, NCH, 512).transpose(2, 1, 0, 3))

    # rope tables: (128 rows = 2 heads x [first32|last32]) x T tokens
    cos = np.asarray(rot_cos, np.float32)[0, :, 0, 0, :HD // 2]  # (S, 32)
    sin = np.asarray(rot_sin, np.float32)[0, :, 0, 0, :HD // 2]
    cT = np.concatenate([cos, cos], 1).T          # (64, S)
    sT = np.concatenate([-sin, sin], 1).T         # (64, S) sign-folded
    cT = np.tile(cT, (2, B))                      # (128, T)
    sT = np.tile(sT, (2, B))
    tab = _bf(np.stack([cT, sT]))

    wqkv_eff = np.asarray(w_qkv, np.float32) * np.asarray(ln1_w, np.float32)[None, :]
    w1_eff = np.asarray(w_mlp1, np.float32) * np.asarray(ln2_w, np.float32)[None, :]
    w_out_f = np.asarray(w_out, np.float32)
    w2_f = np.asarray(w_mlp2, np.float32)

    woutT = w_out_f.T  # (d_in=head dims, e)
    wout_blk = _bf(woutT.reshape(8, 128, 8, 128).transpose(2, 1, 0, 3))  # [m, p, kk, e]
    w1T = w1_eff.T     # (D, 4D)
    w1_blk = _bf(w1T.reshape(8, 128, 32, 128).transpose(2, 1, 0, 3))
    w1_rs = _bf(w1_eff.sum(1).reshape(32, 1, 128))
    w2T = w2_f.T       # (4D, D)
    w2_blk = _bf(w2T.reshape(32, 128, 8, 128).transpose(2, 1, 0, 3))
    b1_arr = _f32(np.asarray(b_mlp1, np.float32).reshape(32, 128, 1))
    b2_arr = _f32(np.asarray(b_mlp2, np.float32).reshape(8, 128, 1))

    in_maps = []
    for c in range(N_CORES):
        w_sl = np.concatenate(
            [wqkv_eff[0 * D + 2 * c * HD: 0 * D + 2 * (c + 1) * HD] * 0.125,
             wqkv_eff[1 * D + 2 * c * HD: 1 * D + 2 * (c + 1) * HD],
             wqkv_eff[2 * D + 2 * c * HD: 2 * D + 2 * (c + 1) * HD]], 0)  # (384, D)
        wT_sl = w_sl.T  # (D, 384) -> [m, p, kk, e]
        wqkv_b = _bf(wT_sl.reshape(8, 128, 3, 128).transpose(2, 1, 0, 3))
        wqkv_rsum = _bf(w_sl.sum(1).reshape(3, 1, 128))
        in_maps.append({
            "x_td": x_td,
            "xT_blk": xT_blk,
            "xT_own": _f32(xT[:, c * TOK:(c + 1) * TOK]),
            "wqkv_blk": wqkv_b,
            "wqkv_rs": wqkv_rsum,
            "tab": tab,
            "wout_blk": wout_blk,
            "w1_blk": w1_blk,
            "w1_rs": w1_rs,
            "b1_t": b1_arr,
            "w2_blk": w2_blk,
            "b2_t": b2_arr,
        })
    return in_maps


def _assemble(results):
    outT = np.concatenate([results[c]["out"] for c in range(N_CORES)], axis=1)
    return np.ascontiguousarray(outT.T.astype(np.float32)).reshape(B, S, D)


def run_spmd(in_maps, **kwargs):
    nc = _get_nc()
    return run_bass_kernel_spmd(nc, in_maps, core_ids=list(range(N_CORES)), **kwargs)


def kernel(x, rot_cos, rot_sin, ln1_w, w_qkv, w_out, ln2_w, w_mlp1, b_mlp1,
           w_mlp2, b_mlp2):
    in_maps = _prep_inputs(x, rot_cos, rot_sin, ln1_w, w_qkv, w_out, ln2_w,
                           w_mlp1, b_mlp1, w_mlp2, b_mlp2)
    res = run_spmd(in_maps)
    return _assemble(res.results)


# revision 16
# speedup vs baseline: 1.5131x; 1.1049x over previous
"""Fused DDiT transformer block (causal) on 8 TRN2 NeuronCores.

Sharding: attention is head-parallel (2 heads/core, 16 total) with QKV
column-sliced per core; an AllToAll then re-shards from head-split to
token-split, and out-proj + MLP run token-parallel (512 tokens/core).
LayerNorm gains are folded into the following matmul weights on the host;
LN centering is folded into the matmuls via an appended K=1 rank-1 update
(-mu[t] * rowsum_w[e]) and the 1/std factor is folded into the RoPE tables
(q,k), a PSUM-eviction multiply (v), or a broadcast multiply (MLP).
Compute dtype bf16 (fp32 accumulation); the residual stream stays fp32.
"""
import sys

for _p in ("/opt/trn_rl_repo",):
    if _p not in sys.path:
        sys.path.append(_p)

import numpy as np
import ml_dtypes

import concourse.bass as bass
import concourse.tile as tile
import concourse.mybir as mybir
from concourse.bass_utils import run_bass_kernel_spmd
from concourse.masks import make_identity

bf16 = mybir.dt.bfloat16
f32 = mybir.dt.float32
AF = mybir.ActivationFunctionType
OP = mybir.AluOpType

N_CORES = 8
B, S, D = 2, 2048, 1024
T = B * S            # 4096 tokens total
NH, HD = 16, 64      # heads, head dim
HPC = NH // N_CORES  # 2 heads per core
TOK = T // N_CORES   # 512 tokens per core in the token-split phase
NT = T // 128        # 32 token tiles of 128
NCH = T // 512       # 8 chunks of 512 tokens
LN_EPS = 1e-5

# ---------------------------------------------------------------------------
# Sync legalizer: this walrus build accepts only ONE sync wait and ONE sync
# update per TPB instruction. Move extras onto same-engine NoOps (engines
# complete instructions in program order, so semantics are preserved).
# ---------------------------------------------------------------------------
_uid = [0]


def _legalize_sync(nc):
    for f in nc.m.functions:
        for bb in f.blocks:
            out = []
            changed = False
            for inst in bb.instructions:
                si = inst.sync_info
                if si is None:
                    out.append(inst)
                    continue
                waits = list(si.on_wait) if si.on_wait else []
                updates = list(si.on_update) if si.on_update else []
                if len(waits) <= 1 and len(updates) <= 1:
                    out.append(inst)
                    continue
                changed = True
                for w in waits[:-1]:
                    _uid[0] += 1
                    nop = mybir.InstNoOp(name=f"syncw-{_uid[0]}", ins=[], outs=[])
                    nop.engine = inst.engine
                    nop.sync_info = mybir.SyncInfo(on_wait=[w], on_update=[])
                    out.append(nop)
                inst.sync_info = mybir.SyncInfo(
                    on_wait=waits[-1:], on_update=updates[:1]
                )
                out.append(inst)
                for u in updates[1:]:
                    _uid[0] += 1
                    nop = mybir.InstNoOp(name=f"syncu-{_uid[0]}", ins=[], outs=[])
                    nop.engine = inst.engine
                    nop.sync_info = mybir.SyncInfo(on_wait=[], on_update=[u])
                    out.append(nop)
            if changed:
                bb.instructions = out
    return nc


# ---------------------------------------------------------------------------
# Kernel graph
# ---------------------------------------------------------------------------
def _build():
    nc = bass.Bass()

    # -- external inputs (per core)
    xT_blk = nc.dram_tensor("xT_blk", (NCH, 128, 8, 512), bf16, kind="ExternalInput")
    xT_own = nc.dram_tensor("xT_own", (D, TOK), f32, kind="ExternalInput")
    wqkv_blk = nc.dram_tensor("wqkv_blk", (3, 128, 8, 128), bf16, kind="ExternalInput")
    wqkv_rs = nc.dram_tensor("wqkv_rs", (3, 1, 128), bf16, kind="ExternalInput")
    tab = nc.dram_tensor("tab", (2, 128, T), bf16, kind="ExternalInput")  # cos, sin(signed)
    wout_blk = nc.dram_tensor("wout_blk", (8, 128, 8, 128), bf16, kind="ExternalInput")
    w1_blk = nc.dram_tensor("w1_blk", (32, 128, 8, 128), bf16, kind="ExternalInput")
    w1_rs = nc.dram_tensor("w1_rs", (32, 1, 128), bf16, kind="ExternalInput")
    b1_t = nc.dram_tensor("b1_t", (32, 128, 1), f32, kind="ExternalInput")
    w2_blk = nc.dram_tensor("w2_blk", (8, 128, 32, 128), bf16, kind="ExternalInput")
    b2_t = nc.dram_tensor("b2_t", (8, 128, 1), f32, kind="ExternalInput")
    out_d = nc.dram_tensor("out", (D, TOK), f32, kind="ExternalOutput")

    # -- internal DRAM
    cc_in = nc.dram_tensor("cc_in", (N_CORES, 128, TOK), bf16, kind="Internal")
    cc_out = nc.dram_tensor("cc_out", (N_CORES, 128, TOK), bf16, kind="Internal")

    with tile.TileContext(nc) as tc, \
         nc.allow_low_precision(reason="bf16 block compute"):
        with tc.tile_pool(name="const", bufs=1) as pconst, \
             tc.tile_pool(name="persist", bufs=1) as pper, \
             tc.tile_pool(name="stream", bufs=2) as pstream, \
             tc.tile_pool(name="big2", bufs=2) as pbig2, \
             tc.tile_pool(name="work", bufs=3) as pwork:
            ident_bf = pconst.tile([128, 128], bf16)
            make_identity(nc, ident_bf)
            mask128 = pconst.tile([128, 128], bf16)
            nc.gpsimd.memset(mask128, 1.0)
            # causal: keep (1.0) where q_local - k_local = f - p >= 0
            nc.gpsimd.affine_select(
                out=mask128, in_=mask128, pattern=[[1, 128]],
                compare_op=OP.is_ge, fill=0.0, base=0, channel_multiplier=-1)
            ones_row = pconst.tile([1, 128], bf16)
            nc.vector.memset(ones_row, 1.0)
            ones_col = pconst.tile([128, 1], bf16)
            nc.vector.memset(ones_col, 1.0)
            ones65 = pconst.tile([65, 64], bf16)
            nc.vector.memset(ones65, 1.0)
            eps_col = pconst.tile([128, 1], f32)
            nc.vector.memset(eps_col, LN_EPS)

            # =============================================================
            # Phase A/B fused: per-chunk LN1 stats on PE + QKV + RoPE + V
            # =============================================================
            attn_pool_cm = tc.tile_pool(name="attn", bufs=1)
            pattn = attn_pool_cm.__enter__()
            negmu_row = pattn.tile([1, T], bf16)
            rstd_row = pattn.tile([1, T], bf16)
            rstd_sb = pattn.tile([128, T], bf16)

            # rope tables (rstd folded in place per chunk; 1/sqrt(hd) in w_q)
            tabs = []
            for ti in range(2):
                raw = pattn.tile([128, T], bf16, name=f"tab{ti}", tag=f"tab{ti}")
                nc.sync.dma_start(out=raw, in_=tab[ti])
                tabs.append(raw)
            tab_c, tab_s = tabs

            # persistent QKV weight tiles (one packed tile per m)
            wq_sb = {}
            for m in range(3):
                w = pconst.tile([128, 8, 128], bf16, name=f"wqkv_{m}", tag=f"wqkv_{m}")
                nc.sync.dma_start(out=w, in_=wqkv_blk[m])
                wq_sb[m] = w
            rs_sb = {}
            for m in range(3):
                r = pconst.tile([1, 128], bf16, name=f"wqkvrs_{m}", tag=f"wqkvrs_{m}")
                nc.sync.dma_start(out=r, in_=wqkv_rs[m])
                rs_sb[m] = r

            # =============================================================
            # Phase B: QKV projection + RoPE + V transpose
            # =============================================================
            qT_sb = pattn.tile([128, T], bf16)
            kT_sb = pattn.tile([128, T], bf16)
            v_all = pattn.tile([128, NT, 130], bf16)

            with tc.tile_pool(name="psQKV", bufs=3, space="PSUM") as psQ, \
                 tc.tile_pool(name="psVT", bufs=1, space="PSUM") as psVT, \
                 tc.tile_pool(name="psST", bufs=1, space="PSUM") as psST:
                for ch in range(NCH):
                    sl = slice(ch * 512, (ch + 1) * 512)
                    xrt = pbig2.tile([128, 8, 512], bf16, tag="xTr")
                    nc.sync.dma_start(out=xrt, in_=xT_blk[ch])
                    # LN1 stats for this chunk via PE column-sum matmuls
                    xsq = pbig2.tile([128, 8, 512], bf16, tag="xsq", bufs=1)
                    nc.scalar.activation(out=xsq, in_=xrt, func=AF.Square)
                    ps_mu = psST.tile([1, 512], f32, tag="mu")
                    ps_sq = psST.tile([1, 512], f32, tag="sq")
                    for kk in range(8):
                        nc.tensor.matmul(ps_mu, ones_col, xrt[:, kk, :],
                                         start=(kk == 0), stop=(kk == 7))
                        nc.tensor.matmul(ps_sq, ones_col, xsq[:, kk, :],
                                         start=(kk == 0), stop=(kk == 7))
                    mu_r = pwork.tile([1, 512], f32, tag="mu1_r", bufs=2)
                    nc.scalar.activation(out=mu_r, in_=ps_mu, func=AF.Copy, scale=1.0 / D)
                    nc.scalar.activation(out=negmu_row[0:1, sl], in_=ps_mu,
                                         func=AF.Copy, scale=-1.0 / D)
                    var_r = pwork.tile([1, 512], f32, tag="var1_r", bufs=2)
                    nc.scalar.activation(out=var_r, in_=ps_sq, func=AF.Copy, scale=1.0 / D)
                    mus_r = pwork.tile([1, 512], f32, tag="mus1_r", bufs=2)
                    nc.vector.tensor_mul(out=mus_r, in0=mu_r, in1=mu_r)
                    nc.vector.tensor_sub(out=var_r, in0=var_r, in1=mus_r)
                    sd_r = pwork.tile([1, 512], f32, tag="sd1_r", bufs=2)
                    nc.scalar.activation(out=sd_r, in_=var_r, func=AF.Sqrt,
                                         bias=eps_col[0:1, :])
                    nc.vector.reciprocal(out=rstd_row[0:1, sl], in_=sd_r)
                    # broadcast rstd; fold into rope tables for this chunk
                    ps_b = psQ.tile([128, 512], f32, tag="bc", bufs=1)
                    nc.tensor.matmul(ps_b, ones_row[0:1, 0:128], rstd_row[0:1, sl],
                                     start=True, stop=True)
                    nc.scalar.activation(out=rstd_sb[:, sl], in_=ps_b, func=AF.Copy)
                    nc.vector.tensor_mul(out=tab_c[:, sl], in0=tab_c[:, sl],
                                         in1=rstd_sb[:, sl])
                    nc.vector.tensor_mul(out=tab_s[:, sl], in0=tab_s[:, sl],
                                         in1=rstd_sb[:, sl])
                    for m in range(3):
                        ps = psQ.tile([128, 512], f32, tag="qkv")
                        for kk in range(8):
                            nc.tensor.matmul(ps, wq_sb[m][:, kk, :], xrt[:, kk, :],
                                             start=(kk == 0), stop=False)
                        nc.tensor.matmul(ps, rs_sb[m], negmu_row[0:1, sl],
                                         start=False, stop=True)
                        if m < 2:  # q or k: rope
                            dst = qT_sb if m == 0 else kT_sb
                            tc_t = pwork.tile([128, 512], bf16, tag="ropec", bufs=2)
                            nc.scalar.activation(out=tc_t, in_=ps, func=AF.Copy)
                            tsw = pwork.tile([128, 512], bf16, tag="ropesw", bufs=2)
                            for h in range(2):
                                for a2 in range(2):
                                    nc.sync.dma_start(
                                        out=tsw[h * 64 + a2 * 32:h * 64 + a2 * 32 + 32, :],
                                        in_=tc_t[h * 64 + (1 - a2) * 32:h * 64 + (1 - a2) * 32 + 32, :])
                            tabc, tabs_ = tab_c, tab_s
                            t1 = pwork.tile([128, 512], bf16, tag="ropet1", bufs=2)
                            nc.vector.tensor_mul(out=t1, in0=tc_t, in1=tabc[:, sl])
                            t2 = pwork.tile([128, 512], bf16, tag="ropet2", bufs=2)
                            nc.vector.tensor_mul(out=t2, in0=tsw, in1=tabs_[:, sl])
                            nc.vector.tensor_add(out=dst[:, sl], in0=t1, in1=t2)
                        else:  # v: scale by rstd, transpose to [t, e] tiles
                            vt = pwork.tile([128, 512], bf16, tag="vtmp")
                            nc.vector.tensor_mul(out=vt, in0=ps, in1=rstd_sb[:, sl])
                            for j in range(4):
                                g = ch * 4 + j
                                pst = psVT.tile([128, 128], bf16, tag="vtr")
                                nc.tensor.transpose(out=pst, in_=vt[:, j * 128:(j + 1) * 128],
                                                    identity=ident_bf)
                                nc.vector.tensor_copy(out=v_all[:, g, 0:64], in_=pst[:, 0:64])
                                nc.vector.tensor_copy(out=v_all[:, g, 65:129], in_=pst[:, 64:128])
                                nc.vector.memset(v_all[:, g, 64:65], 1.0)
                                nc.vector.memset(v_all[:, g, 129:130], 1.0)

            # =============================================================
            # Phase C: causal attention per (batch, head), transposed layout
            # =============================================================
            with tc.tile_pool(name="psSC", bufs=4, space="PSUM") as psSC, \
                 tc.tile_pool(name="psO", bufs=2, space="PSUM") as psO, \
                 tc.tile_pool(name="psBC", bufs=2, space="PSUM") as psBC:
                for b in range(2):
                    for h in range(2):
                        hsl = slice(h * 64, (h + 1) * 64)
                        for qc in range(4):
                            qsl = slice(b * 2048 + qc * 512, b * 2048 + (qc + 1) * 512)
                            nkt = 4 * (qc + 1)
                            ps_o = psO.tile([65, 512], f32, tag="o")
                            for kt in range(nkt):
                                ps_s = psSC.tile([128, 512], f32, tag="sc")
                                ksl = slice(b * 2048 + kt * 128, b * 2048 + (kt + 1) * 128)
                                nc.tensor.matmul(ps_s, kT_sb[hsl, ksl], qT_sb[hsl, qsl],
                                                 start=True, stop=True)
                                p_t = pwork.tile([128, 512], bf16, tag="p", bufs=6)
                                nc.scalar.activation(out=p_t, in_=ps_s, func=AF.Exp)
                                if kt >= 4 * qc:  # diagonal block: causal mask
                                    off = kt * 128 - qc * 512
                                    if off > 0:
                                        nc.vector.memset(p_t[:, 0:off], 0.0)
                                    nc.vector.tensor_mul(
                                        out=p_t[:, off:off + 128],
                                        in0=p_t[:, off:off + 128], in1=mask128)
                                g = b * 16 + kt
                                nc.tensor.matmul(ps_o, v_all[:, g, h * 65:(h + 1) * 65], p_t,
                                                 start=(kt == 0), stop=(kt == nkt - 1))
                            rec = pwork.tile([65, 512], bf16, tag="rec", bufs=2)
                            nc.vector.reciprocal(out=rec[64:65, :], in_=ps_o[64:65, :])
                            ps_b = psBC.tile([64, 512], f32, tag="ob")
                            nc.tensor.matmul(ps_b, ones65[64:65, :], rec[64:65, :],
                                             start=True, stop=True)
                            r64 = pwork.tile([64, 512], bf16, tag="r64", bufs=2)
                            nc.scalar.activation(out=r64, in_=ps_b, func=AF.Copy)
                            o_t = pwork.tile([64, 512], bf16, tag="o_t")
                            nc.vector.tensor_mul(out=o_t, in0=ps_o[0:64, :], in1=r64)
                            j = b * 4 + qc
                            nc.sync.dma_start(
                                out=cc_in[j, h * 64:(h + 1) * 64, :], in_=o_t)

            attn_pool_cm.__exit__(None, None, None)
            mlp_pool_cm = tc.tile_pool(name="mlp", bufs=1)
            pmlp = mlp_pool_cm.__enter__()

            # =============================================================
            # Phase D: AllToAll (head-split -> token-split), out-proj, LN2
            # =============================================================
            nc.gpsimd.collective_compute(
                "AllToAll", OP.bypass, ins=[cc_in[:, :, :]], outs=[cc_out[:, :, :]],
                replica_groups=[list(range(N_CORES))])

            o_own = [pmlp.tile([128, 512], bf16, name=f"oo_{kk}", tag=f"oo_{kk}") for kk in range(8)]
            xT_o = [pmlp.tile([128, 512], f32, name=f"xo_{kk}", tag=f"xo_{kk}") for kk in range(8)]
            for kk in range(8):
                nc.sync.dma_start(out=o_own[kk], in_=cc_out[kk])
                nc.sync.dma_start(out=xT_o[kk], in_=xT_own[kk * 128:(kk + 1) * 128, :])

            xa = [pmlp.tile([128, 512], f32, name=f"xa_{m}", tag=f"xa_{m}") for m in range(8)]
            xab = [pmlp.tile([128, 512], bf16, name=f"xab_{m}", tag=f"xab_{m}") for m in range(8)]
            with tc.tile_pool(name="psOP", bufs=2, space="PSUM") as psOP, \
                 tc.tile_pool(name="psMU", bufs=1, space="PSUM") as psMU, \
                 tc.tile_pool(name="psSQ", bufs=1, space="PSUM") as psSQ, \
                 tc.tile_pool(name="psRB", bufs=1, space="PSUM") as psRB:
                ps_mu = psMU.tile([1, 512], f32)
                ps_sq = psSQ.tile([1, 512], f32)
                for m in range(8):
                    ps = psOP.tile([128, 512], f32, tag="op")
                    w = pstream.tile([128, 8, 128], bf16, tag="wo_st", bufs=3)
                    nc.sync.dma_start(out=w[:, 0:4, :], in_=wout_blk[m, :, 0:4, :])
                    nc.sync.dma_start(out=w[:, 4:8, :], in_=wout_blk[m, :, 4:8, :])
                    for kk in range(8):
                        nc.tensor.matmul(ps, w[:, kk, :], o_own[kk], start=(kk == 0), stop=(kk == 7))
                    nc.vector.tensor_add(out=xa[m], in0=ps, in1=xT_o[m])
                    nc.scalar.activation(out=xab[m], in_=xa[m], func=AF.Copy)
                    sq = pwork.tile([128, 512], bf16, tag="sq")
                    nc.scalar.activation(out=sq, in_=xab[m], func=AF.Square)
                    nc.tensor.matmul(ps_mu, ones_col, xab[m],
                                     start=(m == 0), stop=(m == 7))
                    nc.tensor.matmul(ps_sq, ones_col, sq,
                                     start=(m == 0), stop=(m == 7))

                # LN2 row stats: mu = sum/1024, var = sqsum/1024 - mu^2
                mu_r = pwork.tile([1, 512], f32, tag="mu_r", bufs=1)
                nc.scalar.activation(out=mu_r, in_=ps_mu, func=AF.Copy, scale=1.0 / D)
                var_r = pwork.tile([1, 512], f32, tag="var_r", bufs=1)
                nc.scalar.activation(out=var_r, in_=ps_sq, func=AF.Copy, scale=1.0 / D)
                mus_r = pwork.tile([1, 512], f32, tag="mus_r", bufs=1)
                nc.vector.tensor_mul(out=mus_r, in0=mu_r, in1=mu_r)
                nc.vector.tensor_sub(out=var_r, in0=var_r, in1=mus_r)
                sd_r = pwork.tile([1, 512], f32, tag="sd_r", bufs=1)
                nc.scalar.activation(out=sd_r, in_=var_r, func=AF.Sqrt, bias=eps_col[0:1, :])
                rstd2 = pmlp.tile([1, 512], bf16)
                nc.vector.reciprocal(out=rstd2, in_=sd_r)
                negmu2 = pmlp.tile([1, 512], bf16)
                nc.scalar.activation(out=negmu2, in_=mu_r, func=AF.Copy, scale=-1.0)
                ps_rb = psRB.tile([128, 512], f32)
                nc.tensor.matmul(ps_rb, ones_row[0:1, 0:128], rstd2, start=True, stop=True)
                rstd2_sb = pmlp.tile([128, 512], bf16)
                nc.scalar.activation(out=rstd2_sb, in_=ps_rb, func=AF.Copy)

            # =============================================================
            # Phase E: MLP (token-split, full weights)
            # =============================================================
            u_g = [pmlp.tile([128, 512], bf16, name=f"ug_{m}", tag=f"ug_{m}") for m in range(32)]
            with tc.tile_pool(name="psU", bufs=3, space="PSUM") as psU, \
                 tc.tile_pool(name="psDn", bufs=2, space="PSUM") as psDn:
                for m in range(32):
                    ps = psU.tile([128, 512], f32, tag="u")
                    w = pstream.tile([128, 8, 128], bf16, tag="w1_st", bufs=4)
                    nc.sync.dma_start(out=w[:, 0:4, :], in_=w1_blk[m, :, 0:4, :])
                    nc.sync.dma_start(out=w[:, 4:8, :], in_=w1_blk[m, :, 4:8, :])
                    for kk in range(8):
                        nc.tensor.matmul(ps, w[:, kk, :], xab[kk], start=(kk == 0), stop=False)
                    r1 = pwork.tile([1, 128], bf16, tag="w1rs_st")
                    nc.sync.dma_start(out=r1, in_=w1_rs[m])
                    nc.tensor.matmul(ps, r1, negmu2, start=False, stop=True)
                    upre = pwork.tile([128, 512], bf16, tag="upre")
                    nc.vector.tensor_mul(out=upre, in0=ps, in1=rstd2_sb)
                    b1 = pwork.tile([128, 1], f32, tag="b1_st")
                    nc.sync.dma_start(out=b1, in_=b1_t[m])
                    nc.scalar.activation(out=u_g[m], in_=upre,
                                         func=AF.Gelu_apprx_tanh, bias=b1)
                for m in range(8):
                    ps = psDn.tile([128, 512], f32, tag="dn")
                    w = pstream.tile([128, 32, 128], bf16, tag="w2_st", bufs=2)
                    for q4 in range(4):
                        nc.sync.dma_start(out=w[:, q4 * 8:(q4 + 1) * 8, :],
                                          in_=w2_blk[m, :, q4 * 8:(q4 + 1) * 8, :])
                    for kk in range(32):
                        nc.tensor.matmul(ps, w[:, kk, :], u_g[kk], start=(kk == 0), stop=(kk == 31))
                    b2 = pwork.tile([128, 1], f32, tag="b2_st")
                    nc.sync.dma_start(out=b2, in_=b2_t[m])
                    mt = pwork.tile([128, 512], f32, tag="mt", bufs=2)
                    nc.scalar.activation(out=mt, in_=ps, func=AF.Identity, bias=b2)
                    ot = pwork.tile([128, 512], f32, tag="ot", bufs=2)
                    nc.vector.tensor_add(out=ot, in0=mt, in1=xa[m])
                    nc.sync.dma_start(out=out_d[m * 128:(m + 1) * 128, :], in_=ot)
            mlp_pool_cm.__exit__(None, None, None)

    _legalize_sync(nc)
    return nc


# ---------------------------------------------------------------------------
# Host-side prep + execution
# ---------------------------------------------------------------------------
_NC_CACHE = {}


def _get_nc():
    if "nc" not in _NC_CACHE:
        _NC_CACHE["nc"] = _build()
    return _NC_CACHE["nc"]


def _bf(a):
    return np.ascontiguousarray(a).astype(ml_dtypes.bfloat16)


def _f32(a):
    return np.ascontiguousarray(a, dtype=np.float32)


def _prep_inputs(x, rot_cos, rot_sin, ln1_w, w_qkv, w_out, ln2_w, w_mlp1,
                 b_mlp1, w_mlp2, b_mlp2):
    x = np.asarray(x, np.float32)
    X = x.reshape(T, D)

    xT = X.T  # (D, T)
    # (ch, p, kk, t): partition row p holds all kk-blocks contiguously
    xT_blk = _bf(xT.reshape(8, 128, NCH, 512).transpose(2, 1, 0, 3))

    # rope tables: (128 rows = 2 heads x [first32|last32]) x T tokens
    cos = np.asarray(rot_cos, np.float32)[0, :, 0, 0, :HD // 2]  # (S, 32)
    sin = np.asarray(rot_sin, np.float32)[0, :, 0, 0, :HD // 2]
    cT = np.concatenate([cos, cos], 1).T          # (64, S)
    sT = np.concatenate([-sin, sin], 1).T         # (64, S) sign-folded
    cT = np.tile(cT, (2, B))                      # (128, T)
    sT = np.tile(sT, (2, B))
    tab = _bf(np.stack([cT, sT]))

    wqkv_eff = np.asarray(w_qkv, np.float32) * np.asarray(ln1_w, np.float32)[None, :]
    w1_eff = np.asarray(w_mlp1, np.float32) * np.asarray(ln2_w, np.float32)[None, :]
    w_out_f = np.asarray(w_out, np.float32)
    w2_f = np.asarray(w_mlp2, np.float32)

    woutT = w_out_f.T  # (d_in=head dims, e)
    wout_blk = _bf(woutT.reshape(8, 128, 8, 128).transpose(2, 1, 0, 3))  # [m, p, kk, e]
    w1T = w1_eff.T     # (D, 4D)
    w1_blk = _bf(w1T.reshape(8, 128, 32, 128).transpose(2, 1, 0, 3))
    w1_rs = _bf(w1_eff.sum(1).reshape(32, 1, 128))
    w2T = w2_f.T       # (4D, D)
    w2_blk = _bf(w2T.reshape(32, 128, 8, 128).transpose(2, 1, 0, 3))
    b1_arr = _f32(np.asarray(b_mlp1, np.float32).reshape(32, 128, 1))
    b2_arr = _f32(np.asarray(b_mlp2, np.float32).reshape(8, 128, 1))

    in_maps = []
    for c in range(N_CORES):
        w_sl = np.concatenate(
            [wqkv_eff[0 * D + 2 * c * HD: 0 * D + 2 * (c + 1) * HD] * 0.125,
             wqkv_eff[1 * D + 2 * c * HD: 1 * D + 2 * (c + 1) * HD],
             wqkv_eff[2 * D + 2 * c * HD: 2 * D + 2 * (c + 1) * HD]], 0)  # (384, D)
        wT_sl = w_sl.T  # (D, 384) -> [m, p, kk, e]
        wqkv_b = _bf(wT_sl.reshape(8, 128, 3, 128).transpose(2, 1, 0, 3))
        wqkv_rsum = _bf(w_sl.sum(1).reshape(3, 1, 128))
        in_maps.append({
            "xT_blk": xT_blk,
            "xT_own": _f32(xT[:, c * TOK:(c + 1) * TOK]),
            "wqkv_blk": wqkv_b,
            "wqkv_rs": wqkv_rsum,
            "tab": tab,
            "wout_blk": wout_blk,
            "w1_blk": w1_blk,
            "w1_rs": w1_rs,
            "b1_t": b1_arr,
            "w2_blk": w2_blk,
            "b2_t": b2_arr,
        })
    return in_maps


def _assemble(results):
    outT = np.concatenate([results[c]["out"] for c in range(N_CORES)], axis=1)
    return np.ascontiguousarray(outT.T.astype(np.float32)).reshape(B, S, D)


def run_spmd(in_maps, **kwargs):
    nc = _get_nc()
    return run_bass_kernel_spmd(nc, in_maps, core_ids=list(range(N_CORES)), **kwargs)


def kernel(x, rot_cos, rot_sin, ln1_w, w_qkv, w_out, ln2_w, w_mlp1, b_mlp1,
           w_mlp2, b_mlp2):
    in_maps = _prep_inputs(x, rot_cos, rot_sin, ln1_w, w_qkv, w_out, ln2_w,
                           w_mlp1, b_mlp1, w_mlp2, b_mlp2)
    res = run_spmd(in_maps)
    return _assemble(res.results)


# revision 18
# speedup vs baseline: 1.5253x; 1.0080x over previous
"""Fused DDiT transformer block (causal) on 8 TRN2 NeuronCores.

Sharding: attention is head-parallel (2 heads/core, 16 total) with QKV
column-sliced per core; an AllToAll then re-shards from head-split to
token-split, and out-proj + MLP run token-parallel (512 tokens/core).
LayerNorm gains are folded into the following matmul weights on the host;
LN centering is folded into the matmuls via an appended K=1 rank-1 update
(-mu[t] * rowsum_w[e]) and the 1/std factor is folded into the RoPE tables
(q,k), a PSUM-eviction multiply (v), or a broadcast multiply (MLP).
Compute dtype bf16 (fp32 accumulation); the residual stream stays fp32.
"""
import sys

for _p in ("/opt/trn_rl_repo",):
    if _p not in sys.path:
        sys.path.append(_p)

import numpy as np
import ml_dtypes

import concourse.bass as bass
import concourse.tile as tile
import concourse.mybir as mybir
from concourse.bass_utils import run_bass_kernel_spmd
from concourse.masks import make_identity

bf16 = mybir.dt.bfloat16
f32 = mybir.dt.float32
AF = mybir.ActivationFunctionType
OP = mybir.AluOpType

N_CORES = 8
B, S, D = 2, 2048, 1024
T = B * S            # 4096 tokens total
NH, HD = 16, 64      # heads, head dim
HPC = NH // N_CORES  # 2 heads per core
TOK = T // N_CORES   # 512 tokens per core in the token-split phase
NT = T // 128        # 32 token tiles of 128
NCH = T // 512       # 8 chunks of 512 tokens
LN_EPS = 1e-5

# ---------------------------------------------------------------------------
# Sync legalizer: this walrus build accepts only ONE sync wait and ONE sync
# update per TPB instruction. Move extras onto same-engine NoOps (engines
# complete instructions in program order, so semantics are preserved).
# ---------------------------------------------------------------------------
_uid = [0]


def _legalize_sync(nc):
    for f in nc.m.functions:
        for bb in f.blocks:
            out = []
            changed = False
            for inst in bb.instructions:
                si = inst.sync_info
                if si is None:
                    out.append(inst)
                    continue
                waits = list(si.on_wait) if si.on_wait else []
                updates = list(si.on_update) if si.on_update else []
                if len(waits) <= 1 and len(updates) <= 1:
                    out.append(inst)
                    continue
                changed = True
                for w in waits[:-1]:
                    _uid[0] += 1
                    nop = mybir.InstNoOp(name=f"syncw-{_uid[0]}", ins=[], outs=[])
                    nop.engine = inst.engine
                    nop.sync_info = mybir.SyncInfo(on_wait=[w], on_update=[])
                    out.append(nop)
                inst.sync_info = mybir.SyncInfo(
                    on_wait=waits[-1:], on_update=updates[:1]
                )
                out.append(inst)
                for u in updates[1:]:
                    _uid[0] += 1
                    nop = mybir.InstNoOp(name=f"syncu-{_uid[0]}", ins=[], outs=[])
                    nop.engine = inst.engine
                    nop.sync_info = mybir.SyncInfo(on_wait=[], on_update=[u])
                    out.append(nop)
            if changed:
                bb.instructions = out
    return nc


# ---------------------------------------------------------------------------
# Kernel graph
# ---------------------------------------------------------------------------
def _build():
    nc = bass.Bass()

    # -- external inputs (per core)
    xT_blk = nc.dram_tensor("xT_blk", (NCH, 128, 8, 512), bf16, kind="ExternalInput")
    xT_own = nc.dram_tensor("xT_own", (D, TOK), f32, kind="ExternalInput")
    wqkv_blk = nc.dram_tensor("wqkv_blk", (3, 128, 8, 128), bf16, kind="ExternalInput")
    wqkv_rs = nc.dram_tensor("wqkv_rs", (3, 1, 128), bf16, kind="ExternalInput")
    tab = nc.dram_tensor("tab", (2, 128, T), bf16, kind="ExternalInput")  # cos, sin(signed)
    wout_blk = nc.dram_tensor("wout_blk", (8, 128, 8, 128), bf16, kind="ExternalInput")
    w1_blk = nc.dram_tensor("w1_blk", (32, 128, 8, 128), bf16, kind="ExternalInput")
    w1_rs = nc.dram_tensor("w1_rs", (32, 1, 128), bf16, kind="ExternalInput")
    b1_t = nc.dram_tensor("b1_t", (32, 128, 1), f32, kind="ExternalInput")
    w2_blk = nc.dram_tensor("w2_blk", (8, 128, 32, 128), bf16, kind="ExternalInput")
    b2_t = nc.dram_tensor("b2_t", (8, 128, 1), f32, kind="ExternalInput")
    out_d = nc.dram_tensor("out", (D, TOK), f32, kind="ExternalOutput")

    # -- internal DRAM
    cc_in = nc.dram_tensor("cc_in", (N_CORES, 128, TOK), bf16, kind="Internal")
    cc_out = nc.dram_tensor("cc_out", (N_CORES, 128, TOK), bf16, kind="Internal")

    with tile.TileContext(nc) as tc, \
         nc.allow_low_precision(reason="bf16 block compute"):
        with tc.tile_pool(name="const", bufs=1) as pconst, \
             tc.tile_pool(name="persist", bufs=1) as pper, \
             tc.tile_pool(name="stream", bufs=2) as pstream, \
             tc.tile_pool(name="big2", bufs=2) as pbig2, \
             tc.tile_pool(name="work", bufs=3) as pwork:
            ident_bf = pconst.tile([128, 128], bf16)
            make_identity(nc, ident_bf)
            mask128 = pconst.tile([128, 128], bf16)
            nc.gpsimd.memset(mask128, 1.0)
            # causal: keep (1.0) where q_local - k_local = f - p >= 0
            nc.gpsimd.affine_select(
                out=mask128, in_=mask128, pattern=[[1, 128]],
                compare_op=OP.is_ge, fill=0.0, base=0, channel_multiplier=-1)
            ones_row = pconst.tile([1, 128], bf16)
            nc.vector.memset(ones_row, 1.0)
            ones_col = pconst.tile([128, 1], bf16)
            nc.vector.memset(ones_col, 1.0)
            ones65 = pconst.tile([65, 64], bf16)
            nc.vector.memset(ones65, 1.0)
            eps_col = pconst.tile([128, 1], f32)
            nc.vector.memset(eps_col, LN_EPS)

            # =============================================================
            # Phase A/B fused: per-chunk LN1 stats on PE + QKV + RoPE + V
            # =============================================================
            attn_pool_cm = tc.tile_pool(name="attn", bufs=1)
            pattn = attn_pool_cm.__enter__()
            negmu_row = pattn.tile([1, T], bf16)
            rstd_row = pattn.tile([1, T], bf16)
            rstd_sb = pattn.tile([128, T], bf16)

            # rope tables (rstd folded in place per chunk; 1/sqrt(hd) in w_q)
            tabs = []
            for ti in range(2):
                raw = pattn.tile([128, T], bf16, name=f"tab{ti}", tag=f"tab{ti}")
                nc.sync.dma_start(out=raw, in_=tab[ti])
                tabs.append(raw)
            tab_c, tab_s = tabs

            # persistent QKV weight tiles (one packed tile per m)
            wq_sb = {}
            for m in range(3):
                w = pconst.tile([128, 8, 128], bf16, name=f"wqkv_{m}", tag=f"wqkv_{m}")
                nc.sync.dma_start(out=w, in_=wqkv_blk[m])
                wq_sb[m] = w
            rs_sb = {}
            for m in range(3):
                r = pconst.tile([1, 128], bf16, name=f"wqkvrs_{m}", tag=f"wqkvrs_{m}")
                nc.sync.dma_start(out=r, in_=wqkv_rs[m])
                rs_sb[m] = r

            # =============================================================
            # Phase B: QKV projection + RoPE + V transpose
            # =============================================================
            qT_sb = pattn.tile([128, T], bf16)
            kT_sb = pattn.tile([128, T], bf16)
            v_all = pattn.tile([128, NT, 130], bf16)

            with tc.tile_pool(name="psQKV", bufs=3, space="PSUM") as psQ, \
                 tc.tile_pool(name="psVT", bufs=1, space="PSUM") as psVT, \
                 tc.tile_pool(name="psST", bufs=1, space="PSUM") as psST:
                for ch in range(NCH):
                    sl = slice(ch * 512, (ch + 1) * 512)
                    xrt = pbig2.tile([128, 8, 512], bf16, tag="xTr")
                    nc.sync.dma_start(out=xrt, in_=xT_blk[ch])
                    # LN1 stats for this chunk via PE column-sum matmuls
                    xsq = pbig2.tile([128, 8, 512], bf16, tag="xsq", bufs=1)
                    nc.vector.tensor_mul(out=xsq, in0=xrt, in1=xrt)
                    ps_mu = psST.tile([1, 512], f32, tag="mu")
                    ps_sq = psST.tile([1, 512], f32, tag="sq")
                    for kk in range(8):
                        nc.tensor.matmul(ps_mu, ones_col, xrt[:, kk, :],
                                         start=(kk == 0), stop=(kk == 7))
                        nc.tensor.matmul(ps_sq, ones_col, xsq[:, kk, :],
                                         start=(kk == 0), stop=(kk == 7))
                    nc.vector.tensor_scalar_mul(out=negmu_row[0:1, sl], in0=ps_mu,
                                                scalar1=-1.0 / D)
                    mus_r = pwork.tile([1, 512], f32, tag="mus1_r", bufs=2)
                    nc.vector.tensor_mul(out=mus_r, in0=negmu_row[0:1, sl],
                                         in1=negmu_row[0:1, sl])
                    var_r = pwork.tile([1, 512], f32, tag="var1_r", bufs=2)
                    nc.vector.scalar_tensor_tensor(
                        out=var_r, in0=ps_sq, scalar=1.0 / D, in1=mus_r,
                        op0=OP.mult, op1=OP.subtract)
                    sd_r = pwork.tile([1, 512], f32, tag="sd1_r", bufs=2)
                    nc.scalar.activation(out=sd_r, in_=var_r, func=AF.Sqrt,
                                         bias=eps_col[0:1, :])
                    nc.vector.reciprocal(out=rstd_row[0:1, sl], in_=sd_r)
                    # broadcast rstd; fold into rope tables for this chunk
                    ps_b = psQ.tile([128, 512], f32, tag="bc", bufs=1)
                    nc.tensor.matmul(ps_b, ones_row[0:1, 0:128], rstd_row[0:1, sl],
                                     start=True, stop=True)
                    nc.scalar.activation(out=rstd_sb[:, sl], in_=ps_b, func=AF.Copy)
                    nc.vector.tensor_mul(out=tab_c[:, sl], in0=tab_c[:, sl],
                                         in1=rstd_sb[:, sl])
                    nc.vector.tensor_mul(out=tab_s[:, sl], in0=tab_s[:, sl],
                                         in1=rstd_sb[:, sl])
                    for m in range(3):
                        ps = psQ.tile([128, 512], f32, tag="qkv")
                        for kk in range(8):
                            nc.tensor.matmul(ps, wq_sb[m][:, kk, :], xrt[:, kk, :],
                                             start=(kk == 0), stop=False)
                        nc.tensor.matmul(ps, rs_sb[m], negmu_row[0:1, sl],
                                         start=False, stop=True)
                        if m < 2:  # q or k: rope
                            dst = qT_sb if m == 0 else kT_sb
                            tc_t = pwork.tile([128, 512], bf16, tag="ropec", bufs=2)
                            nc.scalar.activation(out=tc_t, in_=ps, func=AF.Copy)
                            tsw = pwork.tile([128, 512], bf16, tag="ropesw", bufs=2)
                            for h in range(2):
                                for a2 in range(2):
                                    nc.sync.dma_start(
                                        out=tsw[h * 64 + a2 * 32:h * 64 + a2 * 32 + 32, :],
                                        in_=tc_t[h * 64 + (1 - a2) * 32:h * 64 + (1 - a2) * 32 + 32, :])
                            tabc, tabs_ = tab_c, tab_s
                            t1 = pwork.tile([128, 512], bf16, tag="ropet1", bufs=2)
                            nc.vector.tensor_mul(out=t1, in0=tc_t, in1=tabc[:, sl])
                            t2 = pwork.tile([128, 512], bf16, tag="ropet2", bufs=2)
                            nc.vector.tensor_mul(out=t2, in0=tsw, in1=tabs_[:, sl])
                            nc.vector.tensor_add(out=dst[:, sl], in0=t1, in1=t2)
                        else:  # v: scale by rstd, transpose to [t, e] tiles
                            vt = pwork.tile([128, 512], bf16, tag="vtmp")
                            nc.vector.tensor_mul(out=vt, in0=ps, in1=rstd_sb[:, sl])
                            for j in range(4):
                                g = ch * 4 + j
                                pst = psVT.tile([128, 128], bf16, tag="vtr")
                                nc.tensor.transpose(out=pst, in_=vt[:, j * 128:(j + 1) * 128],
                                                    identity=ident_bf)
                                nc.vector.tensor_copy(out=v_all[:, g, 0:64], in_=pst[:, 0:64])
                                nc.vector.tensor_copy(out=v_all[:, g, 65:129], in_=pst[:, 64:128])
                                nc.vector.memset(v_all[:, g, 64:65], 1.0)
                                nc.vector.memset(v_all[:, g, 129:130], 1.0)

            # =============================================================
            # Phase C: causal attention per (batch, head), transposed layout
            # =============================================================
            with tc.tile_pool(name="psSC", bufs=4, space="PSUM") as psSC, \
                 tc.tile_pool(name="psO", bufs=2, space="PSUM") as psO, \
                 tc.tile_pool(name="psBC", bufs=2, space="PSUM") as psBC:
                for b in range(2):
                    for h in range(2):
                        hsl = slice(h * 64, (h + 1) * 64)
                        for qc in range(4):
                            qsl = slice(b * 2048 + qc * 512, b * 2048 + (qc + 1) * 512)
                            nkt = 4 * (qc + 1)
                            ps_o = psO.tile([65, 512], f32, tag="o")
                            for kt in range(nkt):
                                ps_s = psSC.tile([128, 512], f32, tag="sc")
                                ksl = slice(b * 2048 + kt * 128, b * 2048 + (kt + 1) * 128)
                                nc.tensor.matmul(ps_s, kT_sb[hsl, ksl], qT_sb[hsl, qsl],
                                                 start=True, stop=True)
                                p_t = pwork.tile([128, 512], bf16, tag="p", bufs=6)
                                nc.scalar.activation(out=p_t, in_=ps_s, func=AF.Exp)
                                if kt >= 4 * qc:  # diagonal block: causal mask
                                    off = kt * 128 - qc * 512
                                    if off > 0:
                                        nc.vector.memset(p_t[:, 0:off], 0.0)
                                    nc.vector.tensor_mul(
                                        out=p_t[:, off:off + 128],
                                        in0=p_t[:, off:off + 128], in1=mask128)
                                g = b * 16 + kt
                                nc.tensor.matmul(ps_o, v_all[:, g, h * 65:(h + 1) * 65], p_t,
                                                 start=(kt == 0), stop=(kt == nkt - 1))
                            rec = pwork.tile([65, 512], bf16, tag="rec", bufs=2)
                            nc.vector.reciprocal(out=rec[64:65, :], in_=ps_o[64:65, :])
                            ps_b = psBC.tile([64, 512], f32, tag="ob")
                            nc.tensor.matmul(ps_b, ones65[64:65, :], rec[64:65, :],
                                             start=True, stop=True)
                            r64 = pwork.tile([64, 512], bf16, tag="r64", bufs=2)
                            nc.vector.tensor_copy(out=r64, in_=ps_b)
                            o_t = pwork.tile([64, 512], bf16, tag="o_t")
                            nc.vector.tensor_mul(out=o_t, in0=ps_o[0:64, :], in1=r64)
                            j = b * 4 + qc
                            nc.sync.dma_start(
                                out=cc_in[j, h * 64:(h + 1) * 64, :], in_=o_t)

            attn_pool_cm.__exit__(None, None, None)
            mlp_pool_cm = tc.tile_pool(name="mlp", bufs=1)
            pmlp = mlp_pool_cm.__enter__()

            # =============================================================
            # Phase D: AllToAll (head-split -> token-split), out-proj, LN2
            # =============================================================
            nc.gpsimd.collective_compute(
                "AllToAll", OP.bypass, ins=[cc_in[:, :, :]], outs=[cc_out[:, :, :]],
                replica_groups=[list(range(N_CORES))])

            o_own = [pmlp.tile([128, 512], bf16, name=f"oo_{kk}", tag=f"oo_{kk}") for kk in range(8)]
            xT_o = [pmlp.tile([128, 512], f32, name=f"xo_{kk}", tag=f"xo_{kk}") for kk in range(8)]
            for kk in range(8):
                nc.sync.dma_start(out=o_own[kk], in_=cc_out[kk])
                nc.sync.dma_start(out=xT_o[kk], in_=xT_own[kk * 128:(kk + 1) * 128, :])

            xa = [pmlp.tile([128, 512], f32, name=f"xa_{m}", tag=f"xa_{m}") for m in range(8)]
            xab = [pmlp.tile([128, 512], bf16, name=f"xab_{m}", tag=f"xab_{m}") for m in range(8)]
            with tc.tile_pool(name="psOP", bufs=2, space="PSUM") as psOP, \
                 tc.tile_pool(name="psMU", bufs=1, space="PSUM") as psMU, \
                 tc.tile_pool(name="psSQ", bufs=1, space="PSUM") as psSQ, \
                 tc.tile_pool(name="psRB", bufs=1, space="PSUM") as psRB:
                ps_mu = psMU.tile([1, 512], f32)
                ps_sq = psSQ.tile([1, 512], f32)
                for m in range(8):
                    ps = psOP.tile([128, 512], f32, tag="op")
                    w = pstream.tile([128, 8, 128], bf16, tag="wo_st", bufs=3)
                    nc.sync.dma_start(out=w[:, 0:4, :], in_=wout_blk[m, :, 0:4, :])
                    nc.sync.dma_start(out=w[:, 4:8, :], in_=wout_blk[m, :, 4:8, :])
                    for kk in range(8):
                        nc.tensor.matmul(ps, w[:, kk, :], o_own[kk], start=(kk == 0), stop=(kk == 7))
                    nc.vector.tensor_add(out=xa[m], in0=ps, in1=xT_o[m])
                    nc.vector.tensor_copy(out=xab[m], in_=xa[m])
                    sq = pwork.tile([128, 512], bf16, tag="sq", bufs=2)
                    nc.vector.tensor_mul(out=sq, in0=xab[m], in1=xab[m])
                    nc.tensor.matmul(ps_mu, ones_col, xab[m],
                                     start=(m == 0), stop=(m == 7))
                    nc.tensor.matmul(ps_sq, ones_col, sq,
                                     start=(m == 0), stop=(m == 7))

                # LN2 row stats: mu = sum/1024, var = sqsum/1024 - mu^2
                negmu2 = pmlp.tile([1, 512], bf16)
                nc.vector.tensor_scalar_mul(out=negmu2, in0=ps_mu, scalar1=-1.0 / D)
                mus_r = pwork.tile([1, 512], f32, tag="mus_r", bufs=1)
                nc.vector.tensor_mul(out=mus_r, in0=negmu2, in1=negmu2)
                var_r = pwork.tile([1, 512], f32, tag="var_r", bufs=1)
                nc.vector.scalar_tensor_tensor(
                    out=var_r, in0=ps_sq, scalar=1.0 / D, in1=mus_r,
                    op0=OP.mult, op1=OP.subtract)
                sd_r = pwork.tile([1, 512], f32, tag="sd_r", bufs=1)
                nc.scalar.activation(out=sd_r, in_=var_r, func=AF.Sqrt, bias=eps_col[0:1, :])
                rstd2 = pmlp.tile([1, 512], bf16)
                nc.vector.reciprocal(out=rstd2, in_=sd_r)
                ps_rb = psRB.tile([128, 512], f32)
                nc.tensor.matmul(ps_rb, ones_row[0:1, 0:128], rstd2, start=True, stop=True)
                rstd2_sb = pmlp.tile([128, 512], bf16)
                nc.scalar.activation(out=rstd2_sb, in_=ps_rb, func=AF.Copy)

            # =============================================================
            # Phase E: MLP (token-split, full weights)
            # =============================================================
            u_g = [pmlp.tile([128, 512], bf16, name=f"ug_{m}", tag=f"ug_{m}") for m in range(32)]
            with tc.tile_pool(name="psU", bufs=3, space="PSUM") as psU, \
                 tc.tile_pool(name="psDn", bufs=2, space="PSUM") as psDn:
                for m in range(32):
                    ps = psU.tile([128, 512], f32, tag="u")
                    w = pstream.tile([128, 8, 128], bf16, tag="w1_st", bufs=4)
                    nc.sync.dma_start(out=w[:, 0:4, :], in_=w1_blk[m, :, 0:4, :])
                    nc.sync.dma_start(out=w[:, 4:8, :], in_=w1_blk[m, :, 4:8, :])
                    for kk in range(8):
                        nc.tensor.matmul(ps, w[:, kk, :], xab[kk], start=(kk == 0), stop=False)
                    r1 = pwork.tile([1, 128], bf16, tag="w1rs_st")
                    nc.sync.dma_start(out=r1, in_=w1_rs[m])
                    nc.tensor.matmul(ps, r1, negmu2, start=False, stop=True)
                    upre = pwork.tile([128, 512], bf16, tag="upre", bufs=2)
                    nc.vector.tensor_mul(out=upre, in0=ps, in1=rstd2_sb)
                    b1 = pwork.tile([128, 1], f32, tag="b1_st")
                    nc.sync.dma_start(out=b1, in_=b1_t[m])
                    nc.scalar.activation(out=u_g[m], in_=upre,
                                         func=AF.Gelu_apprx_tanh, bias=b1)
                for m in range(8):
                    ps = psDn.tile([128, 512], f32, tag="dn")
                    w = pstream.tile([128, 32, 128], bf16, tag="w2_st", bufs=3)
                    for q4 in range(4):
                        nc.sync.dma_start(out=w[:, q4 * 8:(q4 + 1) * 8, :],
                                          in_=w2_blk[m, :, q4 * 8:(q4 + 1) * 8, :])
                    for kk in range(32):
                        nc.tensor.matmul(ps, w[:, kk, :], u_g[kk], start=(kk == 0), stop=(kk == 31))
                    b2 = pwork.tile([128, 1], f32, tag="b2_st")
                    nc.sync.dma_start(out=b2, in_=b2_t[m])
                    mt = pwork.tile([128, 512], f32, tag="mt", bufs=2)
                    nc.scalar.activation(out=mt, in_=ps, func=AF.Identity, bias=b2)
                    ot = pwork.tile([128, 512], f32, tag="ot", bufs=2)
                    nc.vector.tensor_add(out=ot, in0=mt, in1=xa[m])
                    nc.sync.dma_start(out=out_d[m * 128:(m + 1) * 128, :], in_=ot)
            mlp_pool_cm.__exit__(None, None, None)

    _legalize_sync(nc)
    return nc


# ---------------------------------------------------------------------------
# Host-side prep + execution
# ---------------------------------------------------------------------------
_NC_CACHE = {}


def _get_nc():
    if "nc" not in _NC_CACHE:
        _NC_CACHE["nc"] = _build()
    return _NC_CACHE["nc"]


def _bf(a):
    return np.ascontiguousarray(a).astype(ml_dtypes.bfloat16)


def _f32(a):
    return np.ascontiguousarray(a, dtype=np.float32)


def _prep_inputs(x, rot_cos, rot_sin, ln1_w, w_qkv, w_out, ln2_w, w_mlp1,
                 b_mlp1, w_mlp2, b_mlp2):
    x = np.asarray(x, np.float32)
    X = x.reshape(T, D)

    xT = X.T  # (D, T)
    # (ch, p, kk, t): partition row p holds all kk-blocks contiguously
    xT_blk = _bf(xT.reshape(8, 128, NCH, 512).transpose(2, 1, 0, 3))

    # rope tables: (128 rows = 2 heads x [first32|last32]) x T tokens
    cos = np.asarray(rot_cos, np.float32)[0, :, 0, 0, :HD // 2]  # (S, 32)
    sin = np.asarray(rot_sin, np.float32)[0, :, 0, 0, :HD // 2]
    cT = np.concatenate([cos, cos], 1).T          # (64, S)
    sT = np.concatenate([-sin, sin], 1).T         # (64, S) sign-folded
    cT = np.tile(cT, (2, B))                      # (128, T)
    sT = np.tile(sT, (2, B))
    tab = _bf(np.stack([cT, sT]))

    wqkv_eff = np.asarray(w_qkv, np.float32) * np.asarray(ln1_w, np.float32)[None, :]
    w1_eff = np.asarray(w_mlp1, np.float32) * np.asarray(ln2_w, np.float32)[None, :]
    w_out_f = np.asarray(w_out, np.float32)
    w2_f = np.asarray(w_mlp2, np.float32)

    woutT = w_out_f.T  # (d_in=head dims, e)
    wout_blk = _bf(woutT.reshape(8, 128, 8, 128).transpose(2, 1, 0, 3))  # [m, p, kk, e]
    w1T = w1_eff.T     # (D, 4D)
    w1_blk = _bf(w1T.reshape(8, 128, 32, 128).transpose(2, 1, 0, 3))
    w1_rs = _bf(w1_eff.sum(1).reshape(32, 1, 128))
    w2T = w2_f.T       # (4D, D)
    w2_blk = _bf(w2T.reshape(32, 128, 8, 128).transpose(2, 1, 0, 3))
    b1_arr = _f32(np.asarray(b_mlp1, np.float32).reshape(32, 128, 1))
    b2_arr = _f32(np.asarray(b_mlp2, np.float32).reshape(8, 128, 1))

    in_maps = []
    for c in range(N_CORES):
        w_sl = np.concatenate(
            [wqkv_eff[0 * D + 2 * c * HD: 0 * D + 2 * (c + 1) * HD] * 0.125,
             wqkv_eff[1 * D + 2 * c * HD: 1 * D + 2 * (c + 1) * HD],
             wqkv_eff[2 * D + 2 * c * HD: 2 * D + 2 * (c + 1) * HD]], 0)  # (384, D)
        wT_sl = w_sl.T  # (D, 384) -> [m, p, kk, e]
        wqkv_b = _bf(wT_sl.reshape(8, 128, 3, 128).transpose(2, 1, 0, 3))
        wqkv_rsum = _bf(w_sl.sum(1).reshape(3, 1, 128))
        in_maps.append({
            "xT_blk": xT_blk,
            "xT_own": _f32(xT[:, c * TOK:(c + 1) * TOK]),
            "wqkv_blk": wqkv_b,
            "wqkv_rs": wqkv_rsum,
            "tab": tab,
            "wout_blk": wout_blk,
            "w1_blk": w1_blk,
            "w1_rs": w1_rs,
            "b1_t": b1_arr,
            "w2_blk": w2_blk,
            "b2_t": b2_arr,
        })
    return in_maps


def _assemble(results):
    outT = np.concatenate([results[c]["out"] for c in range(N_CORES)], axis=1)
    return np.ascontiguousarray(outT.T.astype(np.float32)).reshape(B, S, D)


def run_spmd(in_maps, **kwargs):
    nc = _get_nc()
    return run_bass_kernel_spmd(nc, in_maps, core_ids=list(range(N_CORES)), **kwargs)


def kernel(x, rot_cos, rot_sin, ln1_w, w_qkv, w_out, ln2_w, w_mlp1, b_mlp1,
           w_mlp2, b_mlp2):
    in_maps = _prep_inputs(x, rot_cos, rot_sin, ln1_w, w_qkv, w_out, ln2_w,
                           w_mlp1, b_mlp1, w_mlp2, b_mlp2)
    res = run_spmd(in_maps)
    return _assemble(res.results)
